# revision 4
# baseline (speedup 1.0000x reference)
"""Additive (Bahdanau) attention on 8 TRN2 NeuronCores.

Reference computation (B=4, Q=256, K=1024, D=512, H=128):
    qh = q @ w_q.T                      [B,Q,H]
    kh = k @ w_k.T                      [B,K,H]
    scores[b,q,k] = sum_h w_v[h] * tanh(qh[b,q,h] + kh[b,k,h])
    scores masked to -1e6 for k >= valid_lens[b]; softmax over k; out = attn @ v.

Sharding: core c handles batch b = c//2, query rows [(c%2)*128, +128) with ALL
of that batch's valid keys (padded to Kpad = ceil(max(vl)/128)*128). Each core
owns complete softmax rows -> no collectives; host just concatenates.

Masked keys are exact zeros after softmax in fp32 (exp(-1e6) underflows), so
computing only the first Kpad keys reproduces the reference bit-for-bit in
structure. Scores are bounded by sum|w_v|, so softmax needs no max-reduction:
exp(score - C) with constant C is stable.

Score modes:
  direct : tanh computed exactly on ScalarE; H-reduction via per-(q, ktile)
           matmuls with the tanh tile as stationary and w_v as moving.
  fourier: tanh(a+b) ~= sum_m c_m sin(m*w*(a+b)) expanded by the angle-sum
           identity into rank-2M separable features -> the whole score tensor
           becomes TensorE matmuls with contraction dim 2M*128. sin args are
           range-reduced to [-pi, pi] with a fixed-point magic-round + bitmask
           trick (ACT Sin diverges outside one period).
"""
import math
import os
import numpy as np
import ml_dtypes

import concourse.bass as bass
import concourse.mybir as mybir
from concourse.tile import TileContext
from concourse.bass_utils import run_bass_kernel_spmd

F32 = mybir.dt.float32
BF16 = mybir.dt.bfloat16
I32 = mybir.dt.int32
AFT = mybir.ActivationFunctionType
ALU = mybir.AluOpType
BF16NP = ml_dtypes.bfloat16

B, Q, K, D, H = 4, 256, 1024, 512, 128
QC = 128           # query rows per core
N_CORES = 8
MODE = os.environ.get("KMODE", "direct")   # "direct" | "fourier"
M_HARM = 20        # fourier harmonics
FB = 14            # fixed-point fractional bits for sin range reduction
MAGIC = 1.5 * 2.0**23
KEEP_MASK = 0x4B000000 | ((1 << FB) - 1)
ACT_SIN_SCALE = -2.0 * math.pi / (1 << FB)
ACT_SIN_BIAS = 2.0 * math.pi * (2.0**23) / (1 << FB) + math.pi
PAD_BIAS = -30000.0  # exp(score + PAD_BIAS) == 0 exactly for padded keys

_GRAPH_CACHE = {}


# ---------------------------------------------------------------------------
# BIR post-pass: this container's walrus accepts at most 1 sync-wait per
# instruction (2 on EventSemaphore). Tile sometimes emits more (notably the
# kernel-tail drain). Hoist the excess onto standalone EventSemaphores.
def _fix_multiwait(nc):
    ctr = 0
    for f in nc.m.functions:
        for bb in f.blocks:
            ins_list = bb.instructions
            if not any(
                len(i.sync_info.on_wait)
                > (2 if isinstance(i, mybir.InstEventSemaphore) else 1)
                for i in ins_list
                if getattr(i, "sync_info", None) is not None
            ):
                continue
            new_list = []
            for inst in ins_list:
                si = getattr(inst, "sync_info", None)
                if si is not None:
                    waits = list(si.on_wait)
                    cap = 2 if isinstance(inst, mybir.InstEventSemaphore) else 1
                    if len(waits) > cap:
                        extra = waits[cap:]
                        for kk in range(0, len(extra), 2):
                            es = mybir.InstEventSemaphore(
                                name=f"waitfix_{ctr}", engine=inst.engine
                            )
                            ctr += 1
                            es.sync_info = mybir.SyncInfo(
                                on_wait=extra[kk : kk + 2], on_update=[]
                            )
                            new_list.append(es)
                        inst.sync_info = mybir.SyncInfo(
                            on_wait=waits[:cap], on_update=list(si.on_update)
                        )
                new_list.append(inst)
            bb.instructions = new_list
    return nc


def _register_const(nc, val, dtype=F32):
    """Const-AP so activation(bias=<float>) lowers; barrier mirrors Bass init."""
    if (dtype, val) in nc.const_aps.aps:
        return
    t = nc.alloc_sbuf_tensor(f"constap-{len(nc.const_aps.aps)}", [128, 1], dtype)
    nc.gpsimd.memset(t.ap(), val)
    nc.const_aps.aps[(dtype, val)] = t.ap()
    nc.all_engine_barrier()


def _fit_fourier(S):
    """Least-squares fit tanh(s) ~= sum_m c_m sin(m*omega*s) on [-S, S]."""
    P = 1.16 * S
    omega = math.pi / P
    s = np.linspace(-S, S, 20001)
    A = np.sin(np.outer(s, omega * np.arange(1, M_HARM + 1)))
    c, *_ = np.linalg.lstsq(A, np.tanh(s), rcond=None)
    err = float(np.abs(A @ c - np.tanh(s)).max())
    return omega, c.astype(np.float64), err


# ---------------------------------------------------------------------------
def _build_graph(Kpad, mode, omega, cm, c_shift):
    ntk = Kpad // 128
    nc = bass.Bass()
    _register_const(nc, ACT_SIN_BIAS)

    kT_ext = nc.declare_dram_parameter("kT", [D, Kpad], BF16, isOutput=False)
    v_ext = nc.declare_dram_parameter("v", [Kpad, D], BF16, isOutput=False)
    qT_ext = nc.declare_dram_parameter("qT", [D, QC], BF16, isOutput=False)
    wqT_ext = nc.declare_dram_parameter("wqT", [D, H], BF16, isOutput=False)
    wkT_ext = nc.declare_dram_parameter("wkT", [D, H], BF16, isOutput=False)
    maskc_ext = nc.declare_dram_parameter("maskc", [128, ntk], F32, isOutput=False)
    out_ext = nc.declare_dram_parameter("out", [QC, D], F32, isOutput=True)
    if mode == "fourier":
        qsc_ext = nc.declare_dram_parameter("qscale", [H, 2 * M_HARM], F32, isOutput=False)
    else:
        wv_ext = nc.declare_dram_parameter("wv", [H, 1], F32, isOutput=False)

    with TileContext(nc) as tc:
        with tc.tile_pool(name="io", bufs=1) as io, \
             tc.tile_pool(name="work", bufs=1) as work:
            # ---- load inputs
            kT = [io.tile([128, Kpad], BF16, name=f"kT{i}") for i in range(4)]
            for i in range(4):
                nc.sync.dma_start(kT[i][:], kT_ext[bass.ts(i, 128), :])
            v_sb = [io.tile([128, D], BF16, name=f"v{t}") for t in range(ntk)]
            for t in range(ntk):
                nc.sync.dma_start(v_sb[t][:], v_ext[bass.ts(t, 128), :])
            qT = [io.tile([128, QC], BF16, name=f"qT{i}") for i in range(4)]
            wqT = [io.tile([128, H], BF16, name=f"wqT{i}") for i in range(4)]
            wkT = [io.tile([128, H], BF16, name=f"wkT{i}") for i in range(4)]
            for i in range(4):
                nc.sync.dma_start(qT[i][:], qT_ext[bass.ts(i, 128), :])
                nc.sync.dma_start(wqT[i][:], wqT_ext[bass.ts(i, 128), :])
                nc.sync.dma_start(wkT[i][:], wkT_ext[bass.ts(i, 128), :])
            maskc = io.tile([128, ntk], F32)
            nc.sync.dma_start(maskc[:], maskc_ext[:])
            if mode == "fourier":
                qsc = io.tile([H, 2 * M_HARM], F32)
                nc.sync.dma_start(qsc[:], qsc_ext[:])
            else:
                wv = io.tile([H, 1], F32)
                nc.sync.dma_start(wv[:], wv_ext[:])

            # ---- projections kh [H, Kpad], qh [H, QC] (fp32 in SBUF)
            kh_sb = work.tile([H, Kpad], F32)
            qh_sb = work.tile([H, QC], F32)
            with tc.tile_pool(name="psproj", bufs=2, space="PSUM") as psproj:
                for c0 in range(0, Kpad, 512):
                    w = min(512, Kpad - c0)
                    ps = psproj.tile([128, 512], F32, tag="proj")
                    for dt_ in range(4):
                        nc.tensor.matmul(ps[:, :w], wkT[dt_][:],
                                         kT[dt_][:, c0:c0 + w],
                                         start=(dt_ == 0), stop=(dt_ == 3))
                    nc.vector.tensor_copy(kh_sb[:, c0:c0 + w], ps[:, :w])
                ps = psproj.tile([128, 512], F32, tag="proj")
                for dt_ in range(4):
                    nc.tensor.matmul(ps[:, :QC], wqT[dt_][:], qT[dt_][:],
                                     start=(dt_ == 0), stop=(dt_ == 3))
                nc.vector.tensor_copy(qh_sb[:], ps[:, :QC])

            num_tiles = [work.tile([128, QC], BF16, name=f"num{t}") for t in range(ntk)]

            if mode == "fourier":
                _fourier_scores(nc, tc, work, kh_sb, qh_sb, qsc, maskc,
                                num_tiles, Kpad, ntk, omega, cm, c_shift)
            else:
                _direct_scores(nc, tc, work, kh_sb, qh_sb, wv, maskc,
                               num_tiles, Kpad, ntk, c_shift)

            # ---- attn @ v and denominator
            ones = work.tile([128, 1], BF16)
            nc.vector.tensor_copy(ones[:], nc.const_aps.aps[(BF16, 1.0)])
            with tc.tile_pool(name="psout", bufs=1, space="PSUM") as psout, \
                 tc.tile_pool(name="psden", bufs=1, space="PSUM") as psden:
                out_ps = psout.tile([QC, D], F32)
                den_ps = psden.tile([QC, 1], F32)
                for t in range(ntk):
                    nc.tensor.matmul(out_ps[:], num_tiles[t][:], v_sb[t][:],
                                     start=(t == 0), stop=(t == ntk - 1))
                for t in range(ntk):
                    nc.tensor.matmul(den_ps[:], num_tiles[t][:], ones[:, 0:1],
                                     start=(t == 0), stop=(t == ntk - 1))
                recip = work.tile([QC, 1], F32)
                nc.vector.reciprocal(recip[:], den_ps[:])
                out_sb = work.tile([QC, D], F32)
                nc.vector.tensor_scalar(out_sb[:], out_ps[:], recip[:, 0:1],
                                        None, ALU.mult)
                nc.sync.dma_start(out_ext[:], out_sb[:])
    return _fix_multiwait(nc)


def _direct_scores(nc, tc, work, kh_sb, qh_sb, wv, maskc, num_tiles,
                   Kpad, ntk, c_shift):
    """Exact tanh scores. scoresT[k, q] column-by-column via tiny matmuls."""
    with tc.tile_pool(name="pssc", bufs=1, space="PSUM") as pssc, \
         tc.tile_pool(name="tanhp", bufs=3) as tanhp:
        score_ps = [pssc.tile([128, QC], F32, name=f"sc{t}") for t in range(ntk)]
        for qi in range(QC):
            sum_t = tanhp.tile([H, Kpad], F32, tag="sum")
            nc.vector.tensor_scalar(sum_t[:], kh_sb[:], qh_sb[:, qi:qi + 1],
                                    None, ALU.add)
            tanh_t = tanhp.tile([H, Kpad], F32, tag="tanh")
            nc.scalar.activation(tanh_t[:], sum_t[:], AFT.Tanh)
            for t in range(ntk):
                nc.tensor.matmul(score_ps[t][:, qi:qi + 1],
                                 tanh_t[:, bass.ts(t, 128)], wv[:, 0:1],
                                 start=True, stop=True)
        for t in range(ntk):
            nc.scalar.activation(num_tiles[t][:], score_ps[t][:], AFT.Exp,
                                 bias=maskc[:, t:t + 1])


def _fourier_scores(nc, tc, work, kh_sb, qh_sb, qsc, maskc, num_tiles,
                    Kpad, ntk, omega, cm, c_shift):
    """Separable sin-feature scores. Feature j = 2*(m-1)+sc, sc=0 sin / 1 cos.
    G (key) features stationary, Q features (pre-scaled by c_m*w_v) moving;
    G feature j pairs with Q feature of the same m, opposite sc."""
    NF = 2 * M_HARM
    # fixed-point multipliers: t = kh * m/(2P); cos gets +0.25 turn offset
    with tc.tile_pool(name="gfeat", bufs=1) as gfp, \
         tc.tile_pool(name="qfeat", bufs=1) as qfp, \
         tc.tile_pool(name="fwork", bufs=2) as fwp:
        gfeat = gfp.tile([H, NF * Kpad], BF16)      # bank of all G features
        qfs = qfp.tile([H, NF * QC], BF16)          # scaled Q features
        qraw = qfp.tile([H, NF * QC], BF16)

        # ---- G features: u = kh*Cm + (MAGIC [+ 2^(FB-2)]); AND; Sin (chunked)
        G_CHUNK = 5  # features per ACT batch
        for j0 in range(0, NF, G_CHUNK):
            wbuf = fwp.tile([H, G_CHUNK * Kpad], F32, tag="gw")
            for dj in range(G_CHUNK):
                j = j0 + dj
                m = j // 2 + 1
                is_cos = j % 2
                c_fix = m / (2.0 * (math.pi / omega)) * (1 << FB)
                add_c = MAGIC + (2.0 ** (FB - 2) if is_cos else 0.0)
                u = fwp.tile([H, Kpad], F32, tag="gu")
                nc.vector.tensor_scalar(u[:], kh_sb[:], c_fix, add_c,
                                        ALU.mult, ALU.add)
                sl = wbuf[:, dj * Kpad:(dj + 1) * Kpad]
                nc.gpsimd.tensor_scalar(sl.bitcast(I32), u[:].bitcast(I32),
                                        KEEP_MASK, None, ALU.bitwise_and)
            nc.scalar.activation(gfeat[:, j0 * Kpad:(j0 + G_CHUNK) * Kpad],
                                 wbuf[:], AFT.Sin,
                                 scale=ACT_SIN_SCALE, bias=ACT_SIN_BIAS)

        # ---- Q features (then scale by c_m*w_v per feature)
        Q_CHUNK = 20
        for j0 in range(0, NF, Q_CHUNK):
            wbuf = fwp.tile([H, Q_CHUNK * QC], F32, tag="qw")
            for dj in range(Q_CHUNK):
                j = j0 + dj
                m = j // 2 + 1
                is_cos = j % 2
                c_fix = m / (2.0 * (math.pi / omega)) * (1 << FB)
                add_c = MAGIC + (2.0 ** (FB - 2) if is_cos else 0.0)
                u = fwp.tile([H, QC], F32, tag="qu")
                nc.vector.tensor_scalar(u[:], qh_sb[:], c_fix, add_c,
                                        ALU.mult, ALU.add)
                sl = wbuf[:, dj * QC:(dj + 1) * QC]
                nc.gpsimd.tensor_scalar(sl.bitcast(I32), u[:].bitcast(I32),
                                        KEEP_MASK, None, ALU.bitwise_and)
            nc.scalar.activation(qraw[:, j0 * QC:(j0 + Q_CHUNK) * QC],
                                 wbuf[:], AFT.Sin,
                                 scale=ACT_SIN_SCALE, bias=ACT_SIN_BIAS)
        for j in range(NF):
            nc.vector.tensor_scalar(qfs[:, bass.ts(j, QC)],
                                    qraw[:, bass.ts(j, QC)],
                                    qsc[:, j:j + 1], None, ALU.mult)

        # ---- scoresT[k, q] per ktile: accumulate over all NF features
        with tc.tile_pool(name="pssc", bufs=2, space="PSUM") as pssc:
            for t in range(ntk):
                ps = pssc.tile([128, QC], F32, tag="sc")
                for j in range(NF):
                    pj = j ^ 1  # pair: same m, opposite sin/cos
                    nc.tensor.matmul(
                        ps[:],
                        gfeat[:, j * Kpad + t * 128: j * Kpad + (t + 1) * 128],
                        qfs[:, bass.ts(pj, QC)],
                        start=(j == 0), stop=(j == NF - 1))
                nc.scalar.activation(num_tiles[t][:], ps[:], AFT.Exp,
                                     bias=maskc[:, t:t + 1])


# ---------------------------------------------------------------------------
def kernel(q, k, v, valid_lens, w_q, w_k, w_v):
    q = np.asarray(q, np.float32)
    k = np.asarray(k, np.float32)
    v = np.asarray(v, np.float32)
    w_q = np.asarray(w_q, np.float32)
    w_k = np.asarray(w_k, np.float32)
    w_v = np.asarray(w_v, np.float32)
    vls = np.asarray(valid_lens).astype(np.int64)

    Kpad = int(min(K, ((int(vls.max()) + 127) // 128) * 128))
    ntk = Kpad // 128

    # score bound -> constant softmax shift (no max pass needed)
    c_shift = float(np.abs(w_v).sum()) + 0.5

    omega = cm = None
    if MODE == "fourier":
        qh = q.reshape(-1, D) @ w_q.T
        kh = k.reshape(-1, D) @ w_k.T
        S = float(np.abs(qh).max() + np.abs(kh).max()) * 1.02 + 1e-3
        omega, cm, fit_err = _fit_fourier(S)
        assert fit_err < 5e-3, f"fourier fit too coarse: {fit_err}"

    key = (MODE, Kpad, None if omega is None else round(omega, 9),
           None if cm is None else round(float(cm[0]), 9), round(c_shift, 6))
    if key not in _GRAPH_CACHE:
        _GRAPH_CACHE[key] = _build_graph(Kpad, MODE, omega, cm, c_shift)
    nc = _GRAPH_CACHE[key]

    wqT = np.ascontiguousarray(w_q.T).astype(BF16NP)
    wkT = np.ascontiguousarray(w_k.T).astype(BF16NP)
    in_maps = []
    for c in range(N_CORES):
        b = c // 2
        qs = (c % 2) * QC
        maskc = np.full((128, ntk), PAD_BIAS, np.float32)
        vl = int(vls[b])
        for t in range(ntk):
            n_valid = min(128, max(0, vl - t * 128))
            maskc[:n_valid, t] = -c_shift
        im = {
            "kT": np.ascontiguousarray(k[b, :Kpad, :].T).astype(BF16NP),
            "v": np.ascontiguousarray(v[b, :Kpad, :]).astype(BF16NP),
            "qT": np.ascontiguousarray(q[b, qs:qs + QC, :].T).astype(BF16NP),
            "wqT": wqT, "wkT": wkT,
            "maskc": maskc,
        }
        if MODE == "fourier":
            qscale = np.empty((H, 2 * M_HARM), np.float32)
            for j in range(2 * M_HARM):
                qscale[:, j] = w_v * cm[j // 2]
            im["qscale"] = qscale
        else:
            im["wv"] = w_v.reshape(H, 1).astype(np.float32)
        in_maps.append(im)

    res = run_bass_kernel_spmd(nc, in_maps, core_ids=list(range(N_CORES)))
    out = np.empty((B, Q, D), np.float32)
    for c in range(N_CORES):
        b = c // 2
        qs = (c % 2) * QC
        out[b, qs:qs + QC, :] = res.results[c]["out"]
    return out


# revision 5
# speedup vs baseline: 3.6325x; 3.6325x over previous
"""Additive (Bahdanau) attention on 8 TRN2 NeuronCores.

Reference computation (B=4, Q=256, K=1024, D=512, H=128):
    qh = q @ w_q.T                      [B,Q,H]
    kh = k @ w_k.T                      [B,K,H]
    scores[b,q,k] = sum_h w_v[h] * tanh(qh[b,q,h] + kh[b,k,h])
    scores masked to -1e6 for k >= valid_lens[b]; softmax over k; out = attn @ v.

Sharding: core c handles batch b = c//2, query rows [(c%2)*128, +128) with ALL
of that batch's valid keys (padded to Kpad = ceil(max(vl)/128)*128). Each core
owns complete softmax rows -> no collectives; host just concatenates.

Masked keys are exact zeros after softmax in fp32 (exp(-1e6) underflows), so
computing only the first Kpad keys reproduces the reference bit-for-bit in
structure. Scores are bounded by sum|w_v|, so softmax needs no max-reduction:
exp(score - C) with constant C is stable.

Score modes:
  direct : tanh computed exactly on ScalarE; H-reduction via per-(q, ktile)
           matmuls with the tanh tile as stationary and w_v as moving.
  fourier: tanh(a+b) ~= sum_m c_m sin(m*w*(a+b)) expanded by the angle-sum
           identity into rank-2M separable features -> the whole score tensor
           becomes TensorE matmuls with contraction dim 2M*128. sin args are
           range-reduced to [-pi, pi] with a fixed-point magic-round + bitmask
           trick (ACT Sin diverges outside one period).
"""
import math
import os
import numpy as np
import ml_dtypes

import concourse.bass as bass
import concourse.mybir as mybir
from concourse.tile import TileContext
from concourse.bass_utils import run_bass_kernel_spmd

F32 = mybir.dt.float32
BF16 = mybir.dt.bfloat16
I32 = mybir.dt.int32
AFT = mybir.ActivationFunctionType
ALU = mybir.AluOpType
BF16NP = ml_dtypes.bfloat16

B, Q, K, D, H = 4, 256, 1024, 512, 128
QC = 128           # query rows per core
N_CORES = 8
MODE = os.environ.get("KMODE", "direct")   # "direct" | "fourier"
M_HARM = 20        # fourier harmonics
FB = 14            # fixed-point fractional bits for sin range reduction
MAGIC = 1.5 * 2.0**23
KEEP_MASK = 0x4B000000 | ((1 << FB) - 1)
ACT_SIN_SCALE = -2.0 * math.pi / (1 << FB)
ACT_SIN_BIAS = 2.0 * math.pi * (2.0**23) / (1 << FB) + math.pi
PAD_BIAS = -30000.0  # exp(score + PAD_BIAS) == 0 exactly for padded keys

_GRAPH_CACHE = {}


# ---------------------------------------------------------------------------
# BIR post-pass: this container's walrus accepts at most 1 sync-wait per
# instruction (2 on EventSemaphore). Tile sometimes emits more (notably the
# kernel-tail drain). Hoist the excess onto standalone EventSemaphores.
def _fix_multiwait(nc):
    ctr = 0
    for f in nc.m.functions:
        for bb in f.blocks:
            ins_list = bb.instructions
            if not any(
                len(i.sync_info.on_wait)
                > (2 if isinstance(i, mybir.InstEventSemaphore) else 1)
                for i in ins_list
                if getattr(i, "sync_info", None) is not None
            ):
                continue
            new_list = []
            for inst in ins_list:
                si = getattr(inst, "sync_info", None)
                if si is not None:
                    waits = list(si.on_wait)
                    cap = 2 if isinstance(inst, mybir.InstEventSemaphore) else 1
                    if len(waits) > cap:
                        extra = waits[cap:]
                        for kk in range(0, len(extra), 2):
                            es = mybir.InstEventSemaphore(
                                name=f"waitfix_{ctr}", engine=inst.engine
                            )
                            ctr += 1
                            es.sync_info = mybir.SyncInfo(
                                on_wait=extra[kk : kk + 2], on_update=[]
                            )
                            new_list.append(es)
                        inst.sync_info = mybir.SyncInfo(
                            on_wait=waits[:cap], on_update=list(si.on_update)
                        )
                new_list.append(inst)
            bb.instructions = new_list
    return nc


def _register_const(nc, val, dtype=F32):
    """Const-AP so activation(bias=<float>) lowers; barrier mirrors Bass init."""
    if (dtype, val) in nc.const_aps.aps:
        return
    t = nc.alloc_sbuf_tensor(f"constap-{len(nc.const_aps.aps)}", [128, 1], dtype)
    nc.gpsimd.memset(t.ap(), val)
    nc.const_aps.aps[(dtype, val)] = t.ap()
    nc.all_engine_barrier()


def _fit_fourier(S):
    """Least-squares fit tanh(s) ~= sum_m c_m sin(m*omega*s) on [-S, S]."""
    P = 1.16 * S
    omega = math.pi / P
    s = np.linspace(-S, S, 20001)
    A = np.sin(np.outer(s, omega * np.arange(1, M_HARM + 1)))
    c, *_ = np.linalg.lstsq(A, np.tanh(s), rcond=None)
    err = float(np.abs(A @ c - np.tanh(s)).max())
    return omega, c.astype(np.float64), err


# ---------------------------------------------------------------------------
def _build_graph(Kpad, mode, omega, cm, c_shift):
    ntk = Kpad // 128
    nc = bass.Bass()
    _register_const(nc, ACT_SIN_BIAS)

    kT_ext = nc.declare_dram_parameter("kT", [D, Kpad], BF16, isOutput=False)
    v_ext = nc.declare_dram_parameter("v", [Kpad, D], BF16, isOutput=False)
    qT_ext = nc.declare_dram_parameter("qT", [D, QC], BF16, isOutput=False)
    wqT_ext = nc.declare_dram_parameter("wqT", [D, H], BF16, isOutput=False)
    wkT_ext = nc.declare_dram_parameter("wkT", [D, H], BF16, isOutput=False)
    maskc_ext = nc.declare_dram_parameter("maskc", [128, ntk], F32, isOutput=False)
    out_ext = nc.declare_dram_parameter("out", [QC, D], F32, isOutput=True)
    if mode == "fourier":
        qsc_ext = nc.declare_dram_parameter("qscale", [H, 2 * M_HARM], F32, isOutput=False)
    else:
        wv_ext = nc.declare_dram_parameter("wv", [H, 1], F32, isOutput=False)

    with TileContext(nc) as tc:
        with tc.tile_pool(name="io", bufs=1) as io, \
             tc.tile_pool(name="work", bufs=1) as work:
            # ---- load inputs
            kT = [io.tile([128, Kpad], BF16, name=f"kT{i}") for i in range(4)]
            for i in range(4):
                nc.sync.dma_start(kT[i][:], kT_ext[bass.ts(i, 128), :])
            v_sb = [io.tile([128, D], BF16, name=f"v{t}") for t in range(ntk)]
            for t in range(ntk):
                nc.sync.dma_start(v_sb[t][:], v_ext[bass.ts(t, 128), :])
            qT = [io.tile([128, QC], BF16, name=f"qT{i}") for i in range(4)]
            wqT = [io.tile([128, H], BF16, name=f"wqT{i}") for i in range(4)]
            wkT = [io.tile([128, H], BF16, name=f"wkT{i}") for i in range(4)]
            for i in range(4):
                nc.sync.dma_start(qT[i][:], qT_ext[bass.ts(i, 128), :])
                nc.sync.dma_start(wqT[i][:], wqT_ext[bass.ts(i, 128), :])
                nc.sync.dma_start(wkT[i][:], wkT_ext[bass.ts(i, 128), :])
            maskc = io.tile([128, ntk], F32)
            nc.sync.dma_start(maskc[:], maskc_ext[:])
            if mode == "fourier":
                qsc = io.tile([H, 2 * M_HARM], F32)
                nc.sync.dma_start(qsc[:], qsc_ext[:])
            else:
                wv = io.tile([H, 1], F32)
                nc.sync.dma_start(wv[:], wv_ext[:])

            # ---- projections kh [H, Kpad], qh [H, QC] (fp32 in SBUF)
            kh_sb = work.tile([H, Kpad], F32)
            qh_sb = work.tile([H, QC], F32)
            with tc.tile_pool(name="psproj", bufs=2, space="PSUM") as psproj:
                for c0 in range(0, Kpad, 512):
                    w = min(512, Kpad - c0)
                    ps = psproj.tile([128, 512], F32, tag="proj")
                    for dt_ in range(4):
                        nc.tensor.matmul(ps[:, :w], wkT[dt_][:],
                                         kT[dt_][:, c0:c0 + w],
                                         start=(dt_ == 0), stop=(dt_ == 3))
                    nc.vector.tensor_copy(kh_sb[:, c0:c0 + w], ps[:, :w])
                ps = psproj.tile([128, 512], F32, tag="proj")
                for dt_ in range(4):
                    nc.tensor.matmul(ps[:, :QC], wqT[dt_][:], qT[dt_][:],
                                     start=(dt_ == 0), stop=(dt_ == 3))
                nc.vector.tensor_copy(qh_sb[:], ps[:, :QC])

            num_tiles = [work.tile([128, QC], BF16, name=f"num{t}") for t in range(ntk)]

            if mode == "fourier":
                _fourier_scores(nc, tc, work, kh_sb, qh_sb, qsc, maskc,
                                num_tiles, Kpad, ntk, omega, cm, c_shift)
            else:
                _direct_scores(nc, tc, work, kh_sb, qh_sb, wv, maskc,
                               num_tiles, Kpad, ntk, c_shift)

            # ---- attn @ v and denominator
            ones = work.tile([128, 1], BF16)
            nc.vector.tensor_copy(ones[:], nc.const_aps.aps[(BF16, 1.0)])
            with tc.tile_pool(name="psout", bufs=1, space="PSUM") as psout, \
                 tc.tile_pool(name="psden", bufs=1, space="PSUM") as psden:
                out_ps = psout.tile([QC, D], F32)
                den_ps = psden.tile([QC, 1], F32)
                for t in range(ntk):
                    nc.tensor.matmul(out_ps[:], num_tiles[t][:], v_sb[t][:],
                                     start=(t == 0), stop=(t == ntk - 1))
                for t in range(ntk):
                    nc.tensor.matmul(den_ps[:], num_tiles[t][:], ones[:, 0:1],
                                     start=(t == 0), stop=(t == ntk - 1))
                recip = work.tile([QC, 1], F32)
                nc.vector.reciprocal(recip[:], den_ps[:])
                out_sb = work.tile([QC, D], F32)
                nc.vector.tensor_scalar(out_sb[:], out_ps[:], recip[:, 0:1],
                                        None, ALU.mult)
                nc.sync.dma_start(out_ext[:], out_sb[:])
    return _fix_multiwait(nc)


def _direct_scores(nc, tc, work, kh_sb, qh_sb, wv, maskc, num_tiles,
                   Kpad, ntk, c_shift):
    """Exact tanh scores. scoresT[k, q] column-by-column via tiny matmuls."""
    with tc.tile_pool(name="pssc", bufs=1, space="PSUM") as pssc, \
         tc.tile_pool(name="tanhp", bufs=3) as tanhp:
        score_ps = [pssc.tile([128, QC], F32, name=f"sc{t}") for t in range(ntk)]
        for qi in range(QC):
            sum_t = tanhp.tile([H, Kpad], F32, tag="sum")
            nc.vector.tensor_scalar(sum_t[:], kh_sb[:], qh_sb[:, qi:qi + 1],
                                    None, ALU.add)
            tanh_t = tanhp.tile([H, Kpad], F32, tag="tanh")
            nc.scalar.activation(tanh_t[:], sum_t[:], AFT.Tanh)
            for t in range(ntk):
                nc.tensor.matmul(score_ps[t][:, qi:qi + 1],
                                 tanh_t[:, bass.ts(t, 128)], wv[:, 0:1],
                                 start=True, stop=True)
        for t in range(ntk):
            nc.scalar.activation(num_tiles[t][:], score_ps[t][:], AFT.Exp,
                                 bias=maskc[:, t:t + 1])


def _fourier_scores(nc, tc, work, kh_sb, qh_sb, qsc, maskc, num_tiles,
                    Kpad, ntk, omega, cm, c_shift):
    """Separable sin-feature scores. Feature j = 2*(m-1)+sc, sc=0 sin / 1 cos.
    G (key) features stationary, Q features (pre-scaled by c_m*w_v) moving;
    G feature j pairs with Q feature of the same m, opposite sc."""
    NF = 2 * M_HARM
    # fixed-point multipliers: t = kh * m/(2P); cos gets +0.25 turn offset
    with tc.tile_pool(name="gfeat", bufs=1) as gfp, \
         tc.tile_pool(name="qfeat", bufs=1) as qfp, \
         tc.tile_pool(name="fwork", bufs=2) as fwp:
        gfeat = gfp.tile([H, NF * Kpad], BF16)      # bank of all G features
        qfs = qfp.tile([H, NF * QC], BF16)          # scaled Q features
        qraw = qfp.tile([H, NF * QC], BF16)

        # ---- G features: u = kh*Cm + (MAGIC [+ 2^(FB-2)]); AND; Sin (chunked)
        G_CHUNK = 5  # features per ACT batch
        for j0 in range(0, NF, G_CHUNK):
            wbuf = fwp.tile([H, G_CHUNK * Kpad], F32, tag="gw")
            for dj in range(G_CHUNK):
                j = j0 + dj
                m = j // 2 + 1
                is_cos = j % 2
                c_fix = m / (2.0 * (math.pi / omega)) * (1 << FB)
                add_c = MAGIC + (2.0 ** (FB - 2) if is_cos else 0.0)
                u = fwp.tile([H, Kpad], F32, tag="gu")
                nc.vector.tensor_scalar(u[:], kh_sb[:], c_fix, add_c,
                                        ALU.mult, ALU.add)
                sl = wbuf[:, dj * Kpad:(dj + 1) * Kpad]
                nc.vector.tensor_scalar(sl.bitcast(I32), u[:].bitcast(I32),
                                        KEEP_MASK, None, ALU.bitwise_and)
            nc.scalar.activation(gfeat[:, j0 * Kpad:(j0 + G_CHUNK) * Kpad],
                                 wbuf[:], AFT.Sin,
                                 scale=ACT_SIN_SCALE, bias=ACT_SIN_BIAS)

        # ---- Q features (then scale by c_m*w_v per feature)
        Q_CHUNK = 20
        for j0 in range(0, NF, Q_CHUNK):
            wbuf = fwp.tile([H, Q_CHUNK * QC], F32, tag="qw")
            for dj in range(Q_CHUNK):
                j = j0 + dj
                m = j // 2 + 1
                is_cos = j % 2
                c_fix = m / (2.0 * (math.pi / omega)) * (1 << FB)
                add_c = MAGIC + (2.0 ** (FB - 2) if is_cos else 0.0)
                u = fwp.tile([H, QC], F32, tag="qu")
                nc.vector.tensor_scalar(u[:], qh_sb[:], c_fix, add_c,
                                        ALU.mult, ALU.add)
                sl = wbuf[:, dj * QC:(dj + 1) * QC]
                nc.vector.tensor_scalar(sl.bitcast(I32), u[:].bitcast(I32),
                                        KEEP_MASK, None, ALU.bitwise_and)
            nc.scalar.activation(qraw[:, j0 * QC:(j0 + Q_CHUNK) * QC],
                                 wbuf[:], AFT.Sin,
                                 scale=ACT_SIN_SCALE, bias=ACT_SIN_BIAS)
        for j in range(NF):
            nc.vector.tensor_scalar(qfs[:, bass.ts(j, QC)],
                                    qraw[:, bass.ts(j, QC)],
                                    qsc[:, j:j + 1], None, ALU.mult)

        # ---- scoresT[k, q] per ktile: accumulate over all NF features
        with tc.tile_pool(name="pssc", bufs=2, space="PSUM") as pssc:
            for t in range(ntk):
                ps = pssc.tile([128, QC], F32, tag="sc")
                for j in range(NF):
                    pj = j ^ 1  # pair: same m, opposite sin/cos
                    nc.tensor.matmul(
                        ps[:],
                        gfeat[:, j * Kpad + t * 128: j * Kpad + (t + 1) * 128],
                        qfs[:, bass.ts(pj, QC)],
                        start=(j == 0), stop=(j == NF - 1))
                nc.scalar.activation(num_tiles[t][:], ps[:], AFT.Exp,
                                     bias=maskc[:, t:t + 1])


# ---------------------------------------------------------------------------
def kernel(q, k, v, valid_lens, w_q, w_k, w_v):
    q = np.asarray(q, np.float32)
    k = np.asarray(k, np.float32)
    v = np.asarray(v, np.float32)
    w_q = np.asarray(w_q, np.float32)
    w_k = np.asarray(w_k, np.float32)
    w_v = np.asarray(w_v, np.float32)
    vls = np.asarray(valid_lens).astype(np.int64)

    Kpad = int(min(K, ((int(vls.max()) + 127) // 128) * 128))
    ntk = Kpad // 128

    # score bound -> constant softmax shift (no max pass needed)
    c_shift = float(np.abs(w_v).sum()) + 0.5

    omega = cm = None
    if MODE == "fourier":
        qh = q.reshape(-1, D) @ w_q.T
        kh = k.reshape(-1, D) @ w_k.T
        S = float(np.abs(qh).max() + np.abs(kh).max()) * 1.02 + 1e-3
        omega, cm, fit_err = _fit_fourier(S)
        assert fit_err < 5e-3, f"fourier fit too coarse: {fit_err}"

    key = (MODE, Kpad, None if omega is None else round(omega, 9),
           None if cm is None else round(float(cm[0]), 9), round(c_shift, 6))
    if key not in _GRAPH_CACHE:
        _GRAPH_CACHE[key] = _build_graph(Kpad, MODE, omega, cm, c_shift)
    nc = _GRAPH_CACHE[key]

    wqT = np.ascontiguousarray(w_q.T).astype(BF16NP)
    wkT = np.ascontiguousarray(w_k.T).astype(BF16NP)
    in_maps = []
    for c in range(N_CORES):
        b = c // 2
        qs = (c % 2) * QC
        maskc = np.full((128, ntk), PAD_BIAS, np.float32)
        vl = int(vls[b])
        for t in range(ntk):
            n_valid = min(128, max(0, vl - t * 128))
            maskc[:n_valid, t] = -c_shift
        im = {
            "kT": np.ascontiguousarray(k[b, :Kpad, :].T).astype(BF16NP),
            "v": np.ascontiguousarray(v[b, :Kpad, :]).astype(BF16NP),
            "qT": np.ascontiguousarray(q[b, qs:qs + QC, :].T).astype(BF16NP),
            "wqT": wqT, "wkT": wkT,
            "maskc": maskc,
        }
        if MODE == "fourier":
            qscale = np.empty((H, 2 * M_HARM), np.float32)
            for j in range(2 * M_HARM):
                qscale[:, j] = w_v * cm[j // 2]
            im["qscale"] = qscale
        else:
            im["wv"] = w_v.reshape(H, 1).astype(np.float32)
        in_maps.append(im)

    res = run_bass_kernel_spmd(nc, in_maps, core_ids=list(range(N_CORES)))
    out = np.empty((B, Q, D), np.float32)
    for c in range(N_CORES):
        b = c // 2
        qs = (c % 2) * QC
        out[b, qs:qs + QC, :] = res.results[c]["out"]
    return out


# revision 8
# speedup vs baseline: 3.7814x; 1.0410x over previous
"""Additive (Bahdanau) attention on 8 TRN2 NeuronCores.

Reference computation (B=4, Q=256, K=1024, D=512, H=128):
    qh = q @ w_q.T                      [B,Q,H]
    kh = k @ w_k.T                      [B,K,H]
    scores[b,q,k] = sum_h w_v[h] * tanh(qh[b,q,h] + kh[b,k,h])
    scores masked to -1e6 for k >= valid_lens[b]; softmax over k; out = attn @ v.

Sharding: core c handles batch b = c//2, query rows [(c%2)*128, +128) with ALL
of that batch's valid keys (padded to Kpad = ceil(max(vl)/128)*128). Each core
owns complete softmax rows -> no collectives; host just concatenates.

Masked keys are exact zeros after softmax in fp32 (exp(-1e6) underflows), so
computing only the first Kpad keys reproduces the reference bit-for-bit in
structure. Scores are bounded by sum|w_v|, so softmax needs no max-reduction:
exp(score - C) with constant C is stable.

Score modes:
  direct : tanh computed exactly on ScalarE; H-reduction via per-(q, ktile)
           matmuls with the tanh tile as stationary and w_v as moving.
  fourier: tanh(a+b) ~= sum_m c_m sin(m*w*(a+b)) expanded by the angle-sum
           identity into rank-2M separable features -> the whole score tensor
           becomes TensorE matmuls with contraction dim 2M*128. sin args are
           range-reduced to [-pi, pi] with a fixed-point magic-round + bitmask
           trick (ACT Sin diverges outside one period).
"""
import math
import os
import numpy as np
import ml_dtypes

import concourse.bass as bass
import concourse.mybir as mybir
from concourse.tile import TileContext
from concourse.bass_utils import run_bass_kernel_spmd

F32 = mybir.dt.float32
BF16 = mybir.dt.bfloat16
I32 = mybir.dt.int32
AFT = mybir.ActivationFunctionType
ALU = mybir.AluOpType
BF16NP = ml_dtypes.bfloat16

B, Q, K, D, H = 4, 256, 1024, 512, 128
QC = 128           # query rows per core
N_CORES = 8
MODE = os.environ.get("KMODE", "direct")   # "direct" | "fourier"
M_HARM = 20        # fourier harmonics
FB = 14            # fixed-point fractional bits for sin range reduction
MAGIC = 1.5 * 2.0**23
KEEP_MASK = 0x4B000000 | ((1 << FB) - 1)
ACT_SIN_SCALE = -2.0 * math.pi / (1 << FB)
ACT_SIN_BIAS = 2.0 * math.pi * (2.0**23) / (1 << FB) + math.pi
PAD_BIAS = -30000.0  # exp(score + PAD_BIAS) == 0 exactly for padded keys

_GRAPH_CACHE = {}


# ---------------------------------------------------------------------------
# BIR post-pass: this container's walrus accepts at most 1 sync-wait per
# instruction (2 on EventSemaphore). Tile sometimes emits more (notably the
# kernel-tail drain). Hoist the excess onto standalone EventSemaphores.
def _fix_multiwait(nc):
    ctr = 0
    for f in nc.m.functions:
        for bb in f.blocks:
            ins_list = bb.instructions
            if not any(
                len(i.sync_info.on_wait)
                > (2 if isinstance(i, mybir.InstEventSemaphore) else 1)
                for i in ins_list
                if getattr(i, "sync_info", None) is not None
            ):
                continue
            new_list = []
            for inst in ins_list:
                si = getattr(inst, "sync_info", None)
                if si is not None:
                    waits = list(si.on_wait)
                    cap = 2 if isinstance(inst, mybir.InstEventSemaphore) else 1
                    if len(waits) > cap:
                        extra = waits[cap:]
                        for kk in range(0, len(extra), 2):
                            es = mybir.InstEventSemaphore(
                                name=f"waitfix_{ctr}", engine=inst.engine
                            )
                            ctr += 1
                            es.sync_info = mybir.SyncInfo(
                                on_wait=extra[kk : kk + 2], on_update=[]
                            )
                            new_list.append(es)
                        inst.sync_info = mybir.SyncInfo(
                            on_wait=waits[:cap], on_update=list(si.on_update)
                        )
                new_list.append(inst)
            bb.instructions = new_list
    return nc


def _register_const(nc, val, dtype=F32):
    """Const-AP so activation(bias=<float>) lowers; barrier mirrors Bass init."""
    if (dtype, val) in nc.const_aps.aps:
        return
    t = nc.alloc_sbuf_tensor(f"constap-{len(nc.const_aps.aps)}", [128, 1], dtype)
    nc.gpsimd.memset(t.ap(), val)
    nc.const_aps.aps[(dtype, val)] = t.ap()
    nc.all_engine_barrier()


def _fit_fourier(S):
    """Least-squares fit tanh(s) ~= sum_m c_m sin(m*omega*s) on [-S, S]."""
    P = 1.16 * S
    omega = math.pi / P
    s = np.linspace(-S, S, 20001)
    A = np.sin(np.outer(s, omega * np.arange(1, M_HARM + 1)))
    c, *_ = np.linalg.lstsq(A, np.tanh(s), rcond=None)
    err = float(np.abs(A @ c - np.tanh(s)).max())
    return omega, c.astype(np.float64), err


# ---------------------------------------------------------------------------
def _build_graph(Kpad, mode, omega, cm, c_shift):
    ntk = Kpad // 128
    nc = bass.Bass()
    _register_const(nc, ACT_SIN_BIAS)

    kT_ext = nc.declare_dram_parameter("kT", [D, Kpad], BF16, isOutput=False)
    v_ext = nc.declare_dram_parameter("v", [Kpad, D], BF16, isOutput=False)
    qT_ext = nc.declare_dram_parameter("qT", [D, QC], BF16, isOutput=False)
    wqT_ext = nc.declare_dram_parameter("wqT", [D, H], BF16, isOutput=False)
    wkT_ext = nc.declare_dram_parameter("wkT", [D, H], BF16, isOutput=False)
    out_ext = nc.declare_dram_parameter("out", [QC, D], F32, isOutput=True)
    if mode == "fourier":
        qsc_ext = nc.declare_dram_parameter("qscale", [H, 2 * M_HARM], F32, isOutput=False)
        maskr_ext = nc.declare_dram_parameter("maskrow", [1, Kpad], BF16, isOutput=False)
        ident_ext = nc.declare_dram_parameter("ident", [128, 128], BF16, isOutput=False)
    else:
        maskc_ext = nc.declare_dram_parameter("maskc", [128, ntk], F32, isOutput=False)
        wv_ext = nc.declare_dram_parameter("wv", [H, 1], F32, isOutput=False)

    with TileContext(nc) as tc:
        with tc.tile_pool(name="io", bufs=1) as io, \
             tc.tile_pool(name="work", bufs=1) as work:
            # ---- load inputs
            kT = [io.tile([128, Kpad], BF16, name=f"kT{i}") for i in range(4)]
            for i in range(4):
                nc.sync.dma_start(kT[i][:], kT_ext[bass.ts(i, 128), :])
            v_sb = [io.tile([128, D], BF16, name=f"v{t}") for t in range(ntk)]
            for t in range(ntk):
                nc.sync.dma_start(v_sb[t][:], v_ext[bass.ts(t, 128), :])
            qT = [io.tile([128, QC], BF16, name=f"qT{i}") for i in range(4)]
            wqT = [io.tile([128, H], BF16, name=f"wqT{i}") for i in range(4)]
            wkT = [io.tile([128, H], BF16, name=f"wkT{i}") for i in range(4)]
            for i in range(4):
                nc.sync.dma_start(qT[i][:], qT_ext[bass.ts(i, 128), :])
                nc.sync.dma_start(wqT[i][:], wqT_ext[bass.ts(i, 128), :])
                nc.sync.dma_start(wkT[i][:], wkT_ext[bass.ts(i, 128), :])
            if mode == "fourier":
                qsc = io.tile([H, 2 * M_HARM], F32)
                nc.sync.dma_start(qsc[:], qsc_ext[:])
                maskrow = io.tile([1, Kpad], BF16)
                nc.sync.dma_start(maskrow[:], maskr_ext[:])
                ident = io.tile([128, 128], BF16)
                nc.sync.dma_start(ident[:], ident_ext[:])
            else:
                maskc = io.tile([128, ntk], F32)
                nc.sync.dma_start(maskc[:], maskc_ext[:])
                wv = io.tile([H, 1], F32)
                nc.sync.dma_start(wv[:], wv_ext[:])

            # ---- projections kh [H, Kpad], qh [H, QC] (fp32 in SBUF)
            kh_sb = work.tile([H, Kpad], F32)
            qh_sb = work.tile([H, QC], F32)
            with tc.tile_pool(name="psproj", bufs=2, space="PSUM") as psproj:
                for c0 in range(0, Kpad, 512):
                    w = min(512, Kpad - c0)
                    ps = psproj.tile([128, 512], F32, tag="proj")
                    for dt_ in range(4):
                        nc.tensor.matmul(ps[:, :w], wkT[dt_][:],
                                         kT[dt_][:, c0:c0 + w],
                                         start=(dt_ == 0), stop=(dt_ == 3))
                    nc.vector.tensor_copy(kh_sb[:, c0:c0 + w], ps[:, :w])
                ps = psproj.tile([128, 512], F32, tag="proj")
                for dt_ in range(4):
                    nc.tensor.matmul(ps[:, :QC], wqT[dt_][:], qT[dt_][:],
                                     start=(dt_ == 0), stop=(dt_ == 3))
                nc.vector.tensor_copy(qh_sb[:], ps[:, :QC])

            num_tiles = [work.tile([128, QC], BF16, name=f"num{t}") for t in range(ntk)]

            if mode == "fourier":
                _fourier_scores(nc, tc, work, kh_sb, qh_sb, qsc, maskrow, ident,
                                num_tiles, Kpad, ntk, omega, cm, c_shift)
            else:
                _direct_scores(nc, tc, work, kh_sb, qh_sb, wv, maskc,
                               num_tiles, Kpad, ntk, c_shift)

            # ---- attn @ v and denominator
            ones = work.tile([128, 1], BF16)
            nc.vector.tensor_copy(ones[:], nc.const_aps.aps[(BF16, 1.0)])
            with tc.tile_pool(name="psout", bufs=1, space="PSUM") as psout, \
                 tc.tile_pool(name="psden", bufs=1, space="PSUM") as psden:
                out_ps = psout.tile([QC, D], F32)
                den_ps = psden.tile([QC, 1], F32)
                for t in range(ntk):
                    nc.tensor.matmul(out_ps[:], num_tiles[t][:], v_sb[t][:],
                                     start=(t == 0), stop=(t == ntk - 1))
                for t in range(ntk):
                    nc.tensor.matmul(den_ps[:], num_tiles[t][:], ones[:, 0:1],
                                     start=(t == 0), stop=(t == ntk - 1))
                recip = work.tile([QC, 1], F32)
                nc.vector.reciprocal(recip[:], den_ps[:])
                out_sb = work.tile([QC, D], F32)
                nc.vector.tensor_scalar(out_sb[:], out_ps[:], recip[:, 0:1],
                                        None, ALU.mult)
                nc.sync.dma_start(out_ext[:], out_sb[:])
    return _fix_multiwait(nc)


def _direct_scores(nc, tc, work, kh_sb, qh_sb, wv, maskc, num_tiles,
                   Kpad, ntk, c_shift):
    """Exact tanh scores. scoresT[k, q] column-by-column via tiny matmuls."""
    with tc.tile_pool(name="pssc", bufs=1, space="PSUM") as pssc, \
         tc.tile_pool(name="tanhp", bufs=3) as tanhp:
        score_ps = [pssc.tile([128, QC], F32, name=f"sc{t}") for t in range(ntk)]
        for qi in range(QC):
            sum_t = tanhp.tile([H, Kpad], F32, tag="sum")
            nc.vector.tensor_scalar(sum_t[:], kh_sb[:], qh_sb[:, qi:qi + 1],
                                    None, ALU.add)
            tanh_t = tanhp.tile([H, Kpad], F32, tag="tanh")
            nc.scalar.activation(tanh_t[:], sum_t[:], AFT.Tanh)
            for t in range(ntk):
                nc.tensor.matmul(score_ps[t][:, qi:qi + 1],
                                 tanh_t[:, bass.ts(t, 128)], wv[:, 0:1],
                                 start=True, stop=True)
        for t in range(ntk):
            nc.scalar.activation(num_tiles[t][:], score_ps[t][:], AFT.Exp,
                                 bias=maskc[:, t:t + 1])


def _fourier_scores(nc, tc, work, kh_sb, qh_sb, qsc, maskrow, ident,
                    num_tiles, Kpad, ntk, omega, cm, c_shift):
    """Separable sin-feature scores, scores in [q, k] layout.

    kvq = [kh | qh] (fp32, [H, Kpad+QC]): one u/AND pass builds BOTH sides'
    features. Feature j = 2*(m-1)+sc (sc: 0 sin, 1 cos; cos = frac+1/4 in the
    fixed-point domain). Scores psum accumulates over features with Q-side
    (scaled by c_m*w_v) stationary and G-side moving, plus a rank-1 term
    adding maskrow (the -C shift and the -30000 padding mask per key).
    exp -> num [q, k] -> PE-transpose per ktile -> numT tiles for attn."""
    NF = 2 * M_HARM
    W = Kpad + QC
    kvq = work.tile([H, W], F32)
    nc.vector.tensor_copy(kvq[:, :Kpad], kh_sb[:])
    nc.vector.tensor_copy(kvq[:, Kpad:], qh_sb[:])

    with tc.tile_pool(name="featp", bufs=1) as featp, \
         tc.tile_pool(name="fwork", bufs=3) as fwp:
        feats = featp.tile([H, NF * W], BF16)   # [G | Q] per feature slice
        qfs = featp.tile([H, NF * QC], BF16)    # scaled Q-side copies

        CHUNK = 4
        for j0 in range(0, NF, CHUNK):
            wbuf = fwp.tile([H, CHUNK * W], F32, tag="wband", bufs=2)
            for dj in range(CHUNK):
                j = j0 + dj
                m = j // 2 + 1
                c_fix = m / (2.0 * (math.pi / omega)) * (1 << FB)
                add_c = MAGIC + ((1 << FB) / 4.0 if j % 2 else 0.0)
                u = fwp.tile([H, W], F32, tag="u")
                nc.vector.tensor_scalar(u[:], kvq[:], c_fix, add_c,
                                        ALU.mult, ALU.add)
                sl = wbuf[:, dj * W:(dj + 1) * W]
                nc.vector.tensor_scalar(sl.bitcast(I32), u[:].bitcast(I32),
                                        KEEP_MASK, None, ALU.bitwise_and)
            nc.scalar.activation(feats[:, j0 * W:(j0 + CHUNK) * W],
                                 wbuf[:], AFT.Sin,
                                 scale=ACT_SIN_SCALE, bias=ACT_SIN_BIAS)
        for j in range(NF):
            nc.vector.tensor_scalar(qfs[:, bass.ts(j, QC)],
                                    feats[:, j * W + Kpad: (j + 1) * W],
                                    qsc[:, j:j + 1], None, ALU.mult)

        ones_row = work.tile([1, QC], BF16)
        nc.gpsimd.memset(ones_row[:], 1.0)
        num_qk = work.tile([QC, Kpad], BF16)
        with tc.tile_pool(name="pssc", bufs=2, space="PSUM") as pssc:
            for c0 in range(0, Kpad, 512):
                wd = min(512, Kpad - c0)
                ps = pssc.tile([QC, 512], F32, tag="sc")
                # rank-1 mask/shift term first, then feature products
                nc.tensor.matmul(ps[:, :wd], ones_row[:],
                                 maskrow[:, c0:c0 + wd], start=True, stop=False)
                for j in range(NF):
                    pj = j ^ 1
                    nc.tensor.matmul(
                        ps[:, :wd],
                        qfs[:, bass.ts(pj, QC)],
                        feats[:, j * W + c0: j * W + c0 + wd],
                        start=False, stop=(j == NF - 1))
                nc.scalar.activation(num_qk[:, c0:c0 + wd], ps[:, :wd], AFT.Exp)

        # transpose num [q, k] -> numT tiles [k, q]
        with tc.tile_pool(name="pstr", bufs=2, space="PSUM") as pstr:
            for t in range(ntk):
                tr = pstr.tile([128, QC], BF16, tag="tr")
                nc.tensor.transpose(tr[:], num_qk[:, bass.ts(t, 128)], ident[:])
                nc.vector.tensor_copy(num_tiles[t][:], tr[:])


# ---------------------------------------------------------------------------
def kernel(q, k, v, valid_lens, w_q, w_k, w_v):
    q = np.asarray(q, np.float32)
    k = np.asarray(k, np.float32)
    v = np.asarray(v, np.float32)
    w_q = np.asarray(w_q, np.float32)
    w_k = np.asarray(w_k, np.float32)
    w_v = np.asarray(w_v, np.float32)
    vls = np.asarray(valid_lens).astype(np.int64)

    Kpad = int(min(K, ((int(vls.max()) + 127) // 128) * 128))
    ntk = Kpad // 128

    # score bound -> constant softmax shift (no max pass needed)
    c_shift = float(np.abs(w_v).sum()) + 0.5

    omega = cm = None
    if MODE == "fourier":
        qh = q.reshape(-1, D) @ w_q.T
        kh = k.reshape(-1, D) @ w_k.T
        S = float(np.abs(qh).max() + np.abs(kh).max()) * 1.02 + 1e-3
        omega, cm, fit_err = _fit_fourier(S)
        assert fit_err < 5e-3, f"fourier fit too coarse: {fit_err}"

    key = (MODE, Kpad, None if omega is None else round(omega, 9),
           None if cm is None else round(float(cm[0]), 9), round(c_shift, 6))
    if key not in _GRAPH_CACHE:
        _GRAPH_CACHE[key] = _build_graph(Kpad, MODE, omega, cm, c_shift)
    nc = _GRAPH_CACHE[key]

    wqT = np.ascontiguousarray(w_q.T).astype(BF16NP)
    wkT = np.ascontiguousarray(w_k.T).astype(BF16NP)
    in_maps = []
    for c in range(N_CORES):
        b = c // 2
        qs = (c % 2) * QC
        vl = int(vls[b])
        im = {
            "kT": np.ascontiguousarray(k[b, :Kpad, :].T).astype(BF16NP),
            "v": np.ascontiguousarray(v[b, :Kpad, :]).astype(BF16NP),
            "qT": np.ascontiguousarray(q[b, qs:qs + QC, :].T).astype(BF16NP),
            "wqT": wqT, "wkT": wkT,
        }
        if MODE == "fourier":
            qscale = np.empty((H, 2 * M_HARM), np.float32)
            for j in range(2 * M_HARM):
                qscale[:, j] = w_v * cm[j // 2]
            im["qscale"] = qscale
            maskrow = np.full((1, Kpad), PAD_BIAS, np.float32)
            maskrow[0, :vl] = -c_shift
            im["maskrow"] = maskrow.astype(BF16NP)
            im["ident"] = np.eye(128, dtype=BF16NP)
        else:
            maskc = np.full((128, ntk), PAD_BIAS, np.float32)
            for t in range(ntk):
                n_valid = min(128, max(0, vl - t * 128))
                maskc[:n_valid, t] = -c_shift
            im["maskc"] = maskc
            im["wv"] = w_v.reshape(H, 1).astype(np.float32)
        in_maps.append(im)

    res = run_bass_kernel_spmd(nc, in_maps, core_ids=list(range(N_CORES)))
    out = np.empty((B, Q, D), np.float32)
    for c in range(N_CORES):
        b = c // 2
        qs = (c % 2) * QC
        out[b, qs:qs + QC, :] = res.results[c]["out"]
    return out


# revision 10
# speedup vs baseline: 4.3004x; 1.1373x over previous
"""Additive (Bahdanau) attention on 8 TRN2 NeuronCores.

Reference computation (B=4, Q=256, K=1024, D=512, H=128):
    qh = q @ w_q.T                      [B,Q,H]
    kh = k @ w_k.T                      [B,K,H]
    scores[b,q,k] = sum_h w_v[h] * tanh(qh[b,q,h] + kh[b,k,h])
    scores masked to -1e6 for k >= valid_lens[b]; softmax over k; out = attn @ v.

Sharding: core c handles batch b = c//2, query rows [(c%2)*128, +128) with ALL
of that batch's valid keys (padded to Kpad = ceil(max(vl)/128)*128). Each core
owns complete softmax rows -> no collectives; host just concatenates.

Masked keys are exact zeros after softmax in fp32 (exp(-1e6) underflows), so
computing only the first Kpad keys reproduces the reference bit-for-bit in
structure. Scores are bounded by sum|w_v|, so softmax needs no max-reduction:
exp(score - C) with constant C is stable.

Score modes:
  direct : tanh computed exactly on ScalarE; H-reduction via per-(q, ktile)
           matmuls with the tanh tile as stationary and w_v as moving.
  fourier: tanh(a+b) ~= sum_m c_m sin(m*w*(a+b)) expanded by the angle-sum
           identity into rank-2M separable features -> the whole score tensor
           becomes TensorE matmuls with contraction dim 2M*128. sin args are
           range-reduced to [-pi, pi] with a fixed-point magic-round + bitmask
           trick (ACT Sin diverges outside one period).
"""
import math
import os
import numpy as np
import ml_dtypes

import concourse.bass as bass
import concourse.mybir as mybir
from concourse.tile import TileContext
from concourse.bass_utils import run_bass_kernel_spmd

F32 = mybir.dt.float32
BF16 = mybir.dt.bfloat16
I32 = mybir.dt.int32
AFT = mybir.ActivationFunctionType
ALU = mybir.AluOpType
BF16NP = ml_dtypes.bfloat16

B, Q, K, D, H = 4, 256, 1024, 512, 128
QC = 128           # query rows per core
N_CORES = 8
MODE = os.environ.get("KMODE", "direct")   # "direct" | "fourier"
M_HARM = int(os.environ.get("KM", "12"))  # fourier harmonics
FB = 14            # fixed-point fractional bits for sin range reduction
MAGIC = 1.5 * 2.0**23
KEEP_MASK = 0x4B000000 | ((1 << FB) - 1)
ACT_SIN_SCALE = -2.0 * math.pi / (1 << FB)
ACT_SIN_BIAS = 2.0 * math.pi * (2.0**23) / (1 << FB) + math.pi
PAD_BIAS = -30000.0  # exp(score + PAD_BIAS) == 0 exactly for padded keys

_GRAPH_CACHE = {}


# ---------------------------------------------------------------------------
# BIR post-pass: this container's walrus accepts at most 1 sync-wait per
# instruction (2 on EventSemaphore). Tile sometimes emits more (notably the
# kernel-tail drain). Hoist the excess onto standalone EventSemaphores.
def _fix_multiwait(nc):
    ctr = 0
    for f in nc.m.functions:
        for bb in f.blocks:
            ins_list = bb.instructions
            if not any(
                len(i.sync_info.on_wait)
                > (2 if isinstance(i, mybir.InstEventSemaphore) else 1)
                for i in ins_list
                if getattr(i, "sync_info", None) is not None
            ):
                continue
            new_list = []
            for inst in ins_list:
                si = getattr(inst, "sync_info", None)
                if si is not None:
                    waits = list(si.on_wait)
                    cap = 2 if isinstance(inst, mybir.InstEventSemaphore) else 1
                    if len(waits) > cap:
                        extra = waits[cap:]
                        for kk in range(0, len(extra), 2):
                            es = mybir.InstEventSemaphore(
                                name=f"waitfix_{ctr}", engine=inst.engine
                            )
                            ctr += 1
                            es.sync_info = mybir.SyncInfo(
                                on_wait=extra[kk : kk + 2], on_update=[]
                            )
                            new_list.append(es)
                        inst.sync_info = mybir.SyncInfo(
                            on_wait=waits[:cap], on_update=list(si.on_update)
                        )
                new_list.append(inst)
            bb.instructions = new_list
    return nc


def _register_const(nc, val, dtype=F32):
    """Const-AP so activation(bias=<float>) lowers; barrier mirrors Bass init."""
    if (dtype, val) in nc.const_aps.aps:
        return
    t = nc.alloc_sbuf_tensor(f"constap-{len(nc.const_aps.aps)}", [128, 1], dtype)
    nc.gpsimd.memset(t.ap(), val)
    nc.const_aps.aps[(dtype, val)] = t.ap()
    nc.all_engine_barrier()


def _fit_fourier(S):
    """Least-squares fit tanh(s) ~= sum_m c_m sin(m*omega*s) on [-S, S]."""
    P = 1.16 * S
    omega = math.pi / P
    s = np.linspace(-S, S, 20001)
    A = np.sin(np.outer(s, omega * np.arange(1, M_HARM + 1)))
    c, *_ = np.linalg.lstsq(A, np.tanh(s), rcond=None)
    err = float(np.abs(A @ c - np.tanh(s)).max())
    return omega, c.astype(np.float64), err


# ---------------------------------------------------------------------------
def _build_graph(Kpad, mode, omega, cm, c_shift, s_side=0.0):
    ntk = Kpad // 128
    nc = bass.Bass()
    _register_const(nc, ACT_SIN_BIAS)
    _register_const(nc, math.pi / 2)

    kT_ext = nc.declare_dram_parameter("kT", [D, Kpad], BF16, isOutput=False)
    v_ext = nc.declare_dram_parameter("v", [Kpad, D], BF16, isOutput=False)
    qT_ext = nc.declare_dram_parameter("qT", [D, QC], BF16, isOutput=False)
    wqT_ext = nc.declare_dram_parameter("wqT", [D, H], BF16, isOutput=False)
    wkT_ext = nc.declare_dram_parameter("wkT", [D, H], BF16, isOutput=False)
    out_ext = nc.declare_dram_parameter("out", [QC, D], F32, isOutput=True)
    if mode == "fourier":
        qsc_ext = nc.declare_dram_parameter("qscale", [H, 2 * M_HARM], F32, isOutput=False)
        maskr_ext = nc.declare_dram_parameter("maskrow", [1, Kpad], BF16, isOutput=False)
        ident_ext = nc.declare_dram_parameter("ident", [128, 128], BF16, isOutput=False)
    else:
        maskc_ext = nc.declare_dram_parameter("maskc", [128, ntk], F32, isOutput=False)
        wv_ext = nc.declare_dram_parameter("wv", [H, 1], F32, isOutput=False)

    with TileContext(nc) as tc:
        with tc.tile_pool(name="io", bufs=1) as io, \
             tc.tile_pool(name="work", bufs=1) as work:
            # ---- load inputs
            kT = [io.tile([128, Kpad], BF16, name=f"kT{i}") for i in range(4)]
            for i in range(4):
                nc.sync.dma_start(kT[i][:], kT_ext[bass.ts(i, 128), :])
            v_sb = [io.tile([128, D], BF16, name=f"v{t}") for t in range(ntk)]
            for t in range(ntk):
                nc.sync.dma_start(v_sb[t][:], v_ext[bass.ts(t, 128), :])
            qT = [io.tile([128, QC], BF16, name=f"qT{i}") for i in range(4)]
            wqT = [io.tile([128, H], BF16, name=f"wqT{i}") for i in range(4)]
            wkT = [io.tile([128, H], BF16, name=f"wkT{i}") for i in range(4)]
            for i in range(4):
                nc.sync.dma_start(qT[i][:], qT_ext[bass.ts(i, 128), :])
                nc.sync.dma_start(wqT[i][:], wqT_ext[bass.ts(i, 128), :])
                nc.sync.dma_start(wkT[i][:], wkT_ext[bass.ts(i, 128), :])
            if mode == "fourier":
                qsc = io.tile([H, 2 * M_HARM], F32)
                nc.sync.dma_start(qsc[:], qsc_ext[:])
                maskrow = io.tile([1, Kpad], BF16)
                nc.sync.dma_start(maskrow[:], maskr_ext[:])
                ident = io.tile([128, 128], BF16)
                nc.sync.dma_start(ident[:], ident_ext[:])
            else:
                maskc = io.tile([128, ntk], F32)
                nc.sync.dma_start(maskc[:], maskc_ext[:])
                wv = io.tile([H, 1], F32)
                nc.sync.dma_start(wv[:], wv_ext[:])

            # ---- projections kh [H, Kpad], qh [H, QC] (fp32 in SBUF)
            kh_sb = work.tile([H, Kpad], F32)
            qh_sb = work.tile([H, QC], F32)
            with tc.tile_pool(name="psproj", bufs=2, space="PSUM") as psproj:
                for c0 in range(0, Kpad, 512):
                    w = min(512, Kpad - c0)
                    ps = psproj.tile([128, 512], F32, tag="proj")
                    for dt_ in range(4):
                        nc.tensor.matmul(ps[:, :w], wkT[dt_][:],
                                         kT[dt_][:, c0:c0 + w],
                                         start=(dt_ == 0), stop=(dt_ == 3))
                    nc.vector.tensor_copy(kh_sb[:, c0:c0 + w], ps[:, :w])
                ps = psproj.tile([128, 512], F32, tag="proj")
                for dt_ in range(4):
                    nc.tensor.matmul(ps[:, :QC], wqT[dt_][:], qT[dt_][:],
                                     start=(dt_ == 0), stop=(dt_ == 3))
                nc.vector.tensor_copy(qh_sb[:], ps[:, :QC])

            num_tiles = [work.tile([128, QC], BF16, name=f"num{t}") for t in range(ntk)]

            if mode == "fourier":
                _fourier_scores(nc, tc, work, kh_sb, qh_sb, qsc, maskrow, ident,
                                num_tiles, Kpad, ntk, omega, cm, c_shift, s_side)
            else:
                _direct_scores(nc, tc, work, kh_sb, qh_sb, wv, maskc,
                               num_tiles, Kpad, ntk, c_shift)

            # ---- attn @ v and denominator
            ones = work.tile([128, 1], BF16)
            nc.vector.tensor_copy(ones[:], nc.const_aps.aps[(BF16, 1.0)])
            with tc.tile_pool(name="psout", bufs=1, space="PSUM") as psout, \
                 tc.tile_pool(name="psden", bufs=1, space="PSUM") as psden:
                out_ps = psout.tile([QC, D], F32)
                den_ps = psden.tile([QC, 1], F32)
                for t in range(ntk):
                    nc.tensor.matmul(out_ps[:], num_tiles[t][:], v_sb[t][:],
                                     start=(t == 0), stop=(t == ntk - 1))
                for t in range(ntk):
                    nc.tensor.matmul(den_ps[:], num_tiles[t][:], ones[:, 0:1],
                                     start=(t == 0), stop=(t == ntk - 1))
                recip = work.tile([QC, 1], F32)
                nc.vector.reciprocal(recip[:], den_ps[:])
                out_sb = work.tile([QC, D], F32)
                nc.vector.tensor_scalar(out_sb[:], out_ps[:], recip[:, 0:1],
                                        None, ALU.mult)
                nc.sync.dma_start(out_ext[:], out_sb[:])
    return _fix_multiwait(nc)


def _direct_scores(nc, tc, work, kh_sb, qh_sb, wv, maskc, num_tiles,
                   Kpad, ntk, c_shift):
    """Exact tanh scores. scoresT[k, q] column-by-column via tiny matmuls."""
    with tc.tile_pool(name="pssc", bufs=1, space="PSUM") as pssc, \
         tc.tile_pool(name="tanhp", bufs=3) as tanhp:
        score_ps = [pssc.tile([128, QC], F32, name=f"sc{t}") for t in range(ntk)]
        for qi in range(QC):
            sum_t = tanhp.tile([H, Kpad], F32, tag="sum")
            nc.vector.tensor_scalar(sum_t[:], kh_sb[:], qh_sb[:, qi:qi + 1],
                                    None, ALU.add)
            tanh_t = tanhp.tile([H, Kpad], F32, tag="tanh")
            nc.scalar.activation(tanh_t[:], sum_t[:], AFT.Tanh)
            for t in range(ntk):
                nc.tensor.matmul(score_ps[t][:, qi:qi + 1],
                                 tanh_t[:, bass.ts(t, 128)], wv[:, 0:1],
                                 start=True, stop=True)
        for t in range(ntk):
            nc.scalar.activation(num_tiles[t][:], score_ps[t][:], AFT.Exp,
                                 bias=maskc[:, t:t + 1])


def _fourier_scores(nc, tc, work, kh_sb, qh_sb, qsc, maskrow, ident,
                    num_tiles, Kpad, ntk, omega, cm, c_shift, s_side):
    """Separable sin-feature scores, scores accumulated in [q, k] layout.

    kvq = [kh | qh] (fp32, [H, Kpad+QC]): one u/AND pass builds both sides'
    features. Feature j = 2*(m-1)+sc (sc: 0 sin, 1 cos; cos = frac+1/4 in the
    fixed-point domain). Features whose args fit ACT Sin's native [-3.1, 3.1]
    domain skip the fixed-point range reduction entirely.
    Score matmuls are emitted per feature chunk so PE overlaps the VE/ACT
    feature pipeline and stays HAM-warm. A rank-1 (ones x maskrow) term adds
    the softmax shift and the -30000 padding mask. exp -> num [q, k] ->
    PE-transpose per ktile -> numT tiles for the attention matmul."""
    NF = 2 * M_HARM
    W = Kpad + QC
    kvq = work.tile([H, W], F32)
    nc.vector.tensor_copy(kvq[:, :Kpad], kh_sb[:])
    nc.vector.tensor_copy(kvq[:, Kpad:], qh_sb[:])

    ones_row = work.tile([1, QC], BF16)
    nc.gpsimd.memset(ones_row[:], 1.0)

    def feat_params(j):
        m = j // 2 + 1
        is_cos = j % 2
        direct = (m * omega * s_side + (math.pi / 2 if is_cos else 0.0)) < 3.1
        return m, is_cos, direct

    CHUNK = 6
    with tc.tile_pool(name="featp", bufs=1) as featp, \
         tc.tile_pool(name="fwork", bufs=1) as fwp, \
         tc.tile_pool(name="pssc", bufs=1, space="PSUM") as pssc:
        feats = featp.tile([H, NF * W], BF16)   # [G | Q] per feature slice
        qfs = featp.tile([H, NF * QC], BF16)    # scaled Q-side copies

        chunks = [(c0, min(512, Kpad - c0)) for c0 in range(0, Kpad, 512)]
        ps_tiles = [pssc.tile([QC, 512], F32, name=f"sc{i}")
                    for i in range(len(chunks))]
        for i, (c0, wd) in enumerate(chunks):
            nc.tensor.matmul(ps_tiles[i][:, :wd], ones_row[:],
                             maskrow[:, c0:c0 + wd], start=True, stop=False)

        for j0 in range(0, NF, CHUNK):
            js = list(range(j0, min(j0 + CHUNK, NF)))
            red = [j for j in js if not feat_params(j)[2]]
            if red:
                wbuf = fwp.tile([H, len(red) * W], F32, tag="wband", bufs=2)
                for i, j in enumerate(red):
                    m, is_cos, _ = feat_params(j)
                    c_fix = m * omega / (2 * math.pi) * (1 << FB)
                    add_c = MAGIC + ((1 << FB) / 4.0 if is_cos else 0.0)
                    u = fwp.tile([H, W], F32, tag="u", bufs=3)
                    nc.vector.tensor_scalar(u[:], kvq[:], c_fix, add_c,
                                            ALU.mult, ALU.add)
                    nc.vector.tensor_scalar(
                        wbuf[:, i * W:(i + 1) * W].bitcast(I32),
                        u[:].bitcast(I32), KEEP_MASK, None, ALU.bitwise_and)
                # one batched Sin for the chunk's reduced features
                # (non-contiguous dest if chunk mixes direct features; the
                # reduced ones are emitted into their own slices one by one)
                if len(red) == len(js):
                    nc.scalar.activation(
                        feats[:, js[0] * W:(js[-1] + 1) * W], wbuf[:],
                        AFT.Sin, scale=ACT_SIN_SCALE, bias=ACT_SIN_BIAS)
                else:
                    for i, j in enumerate(red):
                        nc.scalar.activation(
                            feats[:, j * W:(j + 1) * W],
                            wbuf[:, i * W:(i + 1) * W],
                            AFT.Sin, scale=ACT_SIN_SCALE, bias=ACT_SIN_BIAS)
            for j in js:
                m, is_cos, direct = feat_params(j)
                if direct:
                    nc.scalar.activation(
                        feats[:, j * W:(j + 1) * W], kvq[:], AFT.Sin,
                        scale=m * omega,
                        bias=(math.pi / 2 if is_cos else 0.0))
            for j in js:
                nc.vector.tensor_scalar(qfs[:, bass.ts(j, QC)],
                                        feats[:, j * W + Kpad:(j + 1) * W],
                                        qsc[:, j:j + 1], None, ALU.mult)
            for j in js:
                pj = j ^ 1
                for i, (c0, wd) in enumerate(chunks):
                    nc.tensor.matmul(
                        ps_tiles[i][:, :wd],
                        qfs[:, bass.ts(pj, QC)],
                        feats[:, j * W + c0: j * W + c0 + wd],
                        start=False, stop=(j == NF - 1))

        num_qk = work.tile([QC, Kpad], BF16)
        for i, (c0, wd) in enumerate(chunks):
            nc.scalar.activation(num_qk[:, c0:c0 + wd], ps_tiles[i][:, :wd],
                                 AFT.Exp)

        # transpose num [q, k] -> numT tiles [k, q]
        with tc.tile_pool(name="pstr", bufs=2, space="PSUM") as pstr:
            for t in range(ntk):
                tr = pstr.tile([128, QC], BF16, tag="tr")
                nc.tensor.transpose(tr[:], num_qk[:, bass.ts(t, 128)], ident[:])
                nc.vector.tensor_copy(num_tiles[t][:], tr[:])


# ---------------------------------------------------------------------------
def kernel(q, k, v, valid_lens, w_q, w_k, w_v):
    q = np.asarray(q, np.float32)
    k = np.asarray(k, np.float32)
    v = np.asarray(v, np.float32)
    w_q = np.asarray(w_q, np.float32)
    w_k = np.asarray(w_k, np.float32)
    w_v = np.asarray(w_v, np.float32)
    vls = np.asarray(valid_lens).astype(np.int64)

    Kpad = int(min(K, ((int(vls.max()) + 127) // 128) * 128))
    ntk = Kpad // 128

    # score bound -> constant softmax shift (no max pass needed)
    c_shift = float(np.abs(w_v).sum()) + 0.5

    omega = cm = None
    s_side = 0.0
    if MODE == "fourier":
        qh = q.reshape(-1, D) @ w_q.T
        kh = k.reshape(-1, D) @ w_k.T
        S = float(np.abs(qh).max() + np.abs(kh).max()) * 1.02 + 1e-3
        s_side = float(max(np.abs(qh).max(), np.abs(kh).max())) * 1.02
        omega, cm, fit_err = _fit_fourier(S)

    key = (MODE, Kpad, None if omega is None else round(omega, 9),
           None if cm is None else round(float(cm[0]), 9), round(c_shift, 6))
    if key not in _GRAPH_CACHE:
        _GRAPH_CACHE[key] = _build_graph(Kpad, MODE, omega, cm, c_shift, s_side)
    nc = _GRAPH_CACHE[key]

    wqT = np.ascontiguousarray(w_q.T).astype(BF16NP)
    wkT = np.ascontiguousarray(w_k.T).astype(BF16NP)
    in_maps = []
    for c in range(N_CORES):
        b = c // 2
        qs = (c % 2) * QC
        vl = int(vls[b])
        im = {
            "kT": np.ascontiguousarray(k[b, :Kpad, :].T).astype(BF16NP),
            "v": np.ascontiguousarray(v[b, :Kpad, :]).astype(BF16NP),
            "qT": np.ascontiguousarray(q[b, qs:qs + QC, :].T).astype(BF16NP),
            "wqT": wqT, "wkT": wkT,
        }
        if MODE == "fourier":
            qscale = np.empty((H, 2 * M_HARM), np.float32)
            for j in range(2 * M_HARM):
                qscale[:, j] = w_v * cm[j // 2]
            im["qscale"] = qscale
            maskrow = np.full((1, Kpad), PAD_BIAS, np.float32)
            maskrow[0, :vl] = -c_shift
            im["maskrow"] = maskrow.astype(BF16NP)
            im["ident"] = np.eye(128, dtype=BF16NP)
        else:
            maskc = np.full((128, ntk), PAD_BIAS, np.float32)
            for t in range(ntk):
                n_valid = min(128, max(0, vl - t * 128))
                maskc[:n_valid, t] = -c_shift
            im["maskc"] = maskc
            im["wv"] = w_v.reshape(H, 1).astype(np.float32)
        in_maps.append(im)

    res = run_bass_kernel_spmd(nc, in_maps, core_ids=list(range(N_CORES)))
    out = np.empty((B, Q, D), np.float32)
    for c in range(N_CORES):
        b = c // 2
        qs = (c % 2) * QC
        out[b, qs:qs + QC, :] = res.results[c]["out"]
    return out


# revision 11
# speedup vs baseline: 5.5278x; 1.2854x over previous
"""Additive (Bahdanau) attention on 8 TRN2 NeuronCores.

Reference computation (B=4, Q=256, K=1024, D=512, H=128):
    qh = q @ w_q.T                      [B,Q,H]
    kh = k @ w_k.T                      [B,K,H]
    scores[b,q,k] = sum_h w_v[h] * tanh(qh[b,q,h] + kh[b,k,h])
    scores masked to -1e6 for k >= valid_lens[b]; softmax over k; out = attn @ v.

Sharding: core c handles batch b = c//2, query rows [(c%2)*128, +128) with ALL
of that batch's valid keys (padded to Kpad = ceil(max(vl)/128)*128). Each core
owns complete softmax rows -> no collectives; host just concatenates.

Masked keys are exact zeros after softmax in fp32 (exp(-1e6) underflows), so
computing only the first Kpad keys reproduces the reference bit-for-bit in
structure. Scores are bounded by sum|w_v|, so softmax needs no max-reduction:
exp(score - C) with constant C is stable.

Score modes:
  direct : tanh computed exactly on ScalarE; H-reduction via per-(q, ktile)
           matmuls with the tanh tile as stationary and w_v as moving.
  fourier: tanh(a+b) ~= sum_m c_m sin(m*w*(a+b)) expanded by the angle-sum
           identity into rank-2M separable features -> the whole score tensor
           becomes TensorE matmuls with contraction dim 2M*128. sin args are
           range-reduced to [-pi, pi] with a fixed-point magic-round + bitmask
           trick (ACT Sin diverges outside one period).
"""
import math
import os
import numpy as np
import ml_dtypes

import concourse.bass as bass
import concourse.mybir as mybir
from concourse.tile import TileContext
from concourse.bass_utils import run_bass_kernel_spmd

F32 = mybir.dt.float32
BF16 = mybir.dt.bfloat16
I32 = mybir.dt.int32
AFT = mybir.ActivationFunctionType
ALU = mybir.AluOpType
BF16NP = ml_dtypes.bfloat16

B, Q, K, D, H = 4, 256, 1024, 512, 128
QC = 128           # query rows per core
N_CORES = 8
MODE = os.environ.get("KMODE", "direct")   # "direct" | "fourier"
M_HARM = int(os.environ.get("KM", "12"))  # fourier harmonics
FB = 14            # fixed-point fractional bits for sin range reduction
MAGIC = 1.5 * 2.0**23
KEEP_MASK = 0x4B000000 | ((1 << FB) - 1)
ACT_SIN_SCALE = -2.0 * math.pi / (1 << FB)
ACT_SIN_BIAS = 2.0 * math.pi * (2.0**23) / (1 << FB) + math.pi
PAD_BIAS = -30000.0  # exp(score + PAD_BIAS) == 0 exactly for padded keys

_GRAPH_CACHE = {}


# ---------------------------------------------------------------------------
# BIR post-pass: this container's walrus accepts at most 1 sync-wait per
# instruction (2 on EventSemaphore). Tile sometimes emits more (notably the
# kernel-tail drain). Hoist the excess onto standalone EventSemaphores.
def _fix_multiwait(nc):
    ctr = 0
    for f in nc.m.functions:
        for bb in f.blocks:
            ins_list = bb.instructions
            if not any(
                len(i.sync_info.on_wait)
                > (2 if isinstance(i, mybir.InstEventSemaphore) else 1)
                for i in ins_list
                if getattr(i, "sync_info", None) is not None
            ):
                continue
            new_list = []
            for inst in ins_list:
                si = getattr(inst, "sync_info", None)
                if si is not None:
                    waits = list(si.on_wait)
                    cap = 2 if isinstance(inst, mybir.InstEventSemaphore) else 1
                    if len(waits) > cap:
                        extra = waits[cap:]
                        for kk in range(0, len(extra), 2):
                            es = mybir.InstEventSemaphore(
                                name=f"waitfix_{ctr}", engine=inst.engine
                            )
                            ctr += 1
                            es.sync_info = mybir.SyncInfo(
                                on_wait=extra[kk : kk + 2], on_update=[]
                            )
                            new_list.append(es)
                        inst.sync_info = mybir.SyncInfo(
                            on_wait=waits[:cap], on_update=list(si.on_update)
                        )
                new_list.append(inst)
            bb.instructions = new_list
    return nc


def _register_const(nc, val, dtype=F32):
    """Const-AP so activation(bias=<float>) lowers; barrier mirrors Bass init."""
    if (dtype, val) in nc.const_aps.aps:
        return
    t = nc.alloc_sbuf_tensor(f"constap-{len(nc.const_aps.aps)}", [128, 1], dtype)
    nc.gpsimd.memset(t.ap(), val)
    nc.const_aps.aps[(dtype, val)] = t.ap()
    nc.all_engine_barrier()


def _fit_fourier(S):
    """Least-squares fit tanh(s) ~= sum_m c_m sin(m*omega*s) on [-S, S]."""
    P = 1.16 * S
    omega = math.pi / P
    s = np.linspace(-S, S, 20001)
    A = np.sin(np.outer(s, omega * np.arange(1, M_HARM + 1)))
    c, *_ = np.linalg.lstsq(A, np.tanh(s), rcond=None)
    err = float(np.abs(A @ c - np.tanh(s)).max())
    return omega, c.astype(np.float64), err


# ---------------------------------------------------------------------------
def _build_graph(Kpad, mode, omega, cm, c_shift, s_side=0.0):
    ntk = Kpad // 128
    nc = bass.Bass()
    _register_const(nc, ACT_SIN_BIAS)
    _register_const(nc, math.pi / 2)

    kT_ext = nc.declare_dram_parameter("kT", [D, Kpad], BF16, isOutput=False)
    v_ext = nc.declare_dram_parameter("v", [Kpad, D], BF16, isOutput=False)
    qT_ext = nc.declare_dram_parameter("qT", [D, QC], BF16, isOutput=False)
    wqT_ext = nc.declare_dram_parameter("wqT", [D, H], BF16, isOutput=False)
    wkT_ext = nc.declare_dram_parameter("wkT", [D, H], BF16, isOutput=False)
    out_ext = nc.declare_dram_parameter("out", [QC, D], F32, isOutput=True)
    if mode == "fourier":
        qsc_ext = nc.declare_dram_parameter("qscale", [H, 2 * M_HARM], F32, isOutput=False)
        maskr_ext = nc.declare_dram_parameter("maskrow", [1, Kpad], BF16, isOutput=False)
        ident_ext = nc.declare_dram_parameter("ident", [128, 128], BF16, isOutput=False)
    else:
        maskc_ext = nc.declare_dram_parameter("maskc", [128, ntk], F32, isOutput=False)
        wv_ext = nc.declare_dram_parameter("wv", [H, 1], F32, isOutput=False)

    with TileContext(nc) as tc:
        with tc.tile_pool(name="io", bufs=1) as io, \
             tc.tile_pool(name="work", bufs=1) as work:
            # ---- load inputs. Small tensors on the gpsimd (SWDGE) queue,
            # big ones on sync (HWDGE); weights/kT first, v last (only the
            # attention tail needs it).
            wqT = [io.tile([128, H], BF16, name=f"wqT{i}") for i in range(4)]
            wkT = [io.tile([128, H], BF16, name=f"wkT{i}") for i in range(4)]
            qT = [io.tile([128, QC], BF16, name=f"qT{i}") for i in range(4)]
            for i in range(4):
                nc.gpsimd.dma_start(wkT[i][:], wkT_ext[bass.ts(i, 128), :])
                nc.gpsimd.dma_start(wqT[i][:], wqT_ext[bass.ts(i, 128), :])
                nc.gpsimd.dma_start(qT[i][:], qT_ext[bass.ts(i, 128), :])
            kT = [io.tile([128, Kpad], BF16, name=f"kT{i}") for i in range(4)]
            for i in range(4):
                nc.sync.dma_start(kT[i][:], kT_ext[bass.ts(i, 128), :])
            if mode == "fourier":
                qsc = io.tile([H, 2 * M_HARM], F32)
                nc.gpsimd.dma_start(qsc[:], qsc_ext[:])
                maskrow = io.tile([1, Kpad], BF16)
                nc.gpsimd.dma_start(maskrow[:], maskr_ext[:])
                ident = io.tile([128, 128], BF16)
                nc.gpsimd.dma_start(ident[:], ident_ext[:])
            else:
                maskc = io.tile([128, ntk], F32)
                nc.gpsimd.dma_start(maskc[:], maskc_ext[:])
                wv = io.tile([H, 1], F32)
                nc.gpsimd.dma_start(wv[:], wv_ext[:])
            v_sb = [io.tile([128, D], BF16, name=f"v{t}") for t in range(ntk)]
            for t in range(ntk):
                nc.sync.dma_start(v_sb[t][:], v_ext[bass.ts(t, 128), :])

            # ---- projections -> kvq = [kh | qh] fp32 (fourier) or
            # kh_sb/qh_sb (direct)
            if mode == "fourier":
                kvq = work.tile([H, Kpad + QC], F32)
                kh_sb = kvq[:, :Kpad]
                qh_sb = kvq[:, Kpad:]
            else:
                kh_t = work.tile([H, Kpad], F32)
                qh_t = work.tile([H, QC], F32)
                kh_sb = kh_t[:]
                qh_sb = qh_t[:]
            with tc.tile_pool(name="psproj", bufs=2, space="PSUM") as psproj:
                for c0 in range(0, Kpad, 512):
                    w = min(512, Kpad - c0)
                    ps = psproj.tile([128, 512], F32, tag="proj")
                    for dt_ in range(4):
                        nc.tensor.matmul(ps[:, :w], wkT[dt_][:],
                                         kT[dt_][:, c0:c0 + w],
                                         start=(dt_ == 0), stop=(dt_ == 3))
                    nc.vector.tensor_copy(kh_sb[:, c0:c0 + w], ps[:, :w])
                ps = psproj.tile([128, 512], F32, tag="proj")
                for dt_ in range(4):
                    nc.tensor.matmul(ps[:, :QC], wqT[dt_][:], qT[dt_][:],
                                     start=(dt_ == 0), stop=(dt_ == 3))
                nc.vector.tensor_copy(qh_sb[:], ps[:, :QC])

            num_tiles = [work.tile([128, QC], BF16, name=f"num{t}") for t in range(ntk)]

            if mode == "fourier":
                _fourier_scores(nc, tc, work, kvq, qsc, maskrow, ident,
                                num_tiles, Kpad, ntk, omega, cm, c_shift, s_side)
            else:
                _direct_scores(nc, tc, work, kh_sb, qh_sb, wv, maskc,
                               num_tiles, Kpad, ntk, c_shift)

            # ---- attn @ v and denominator
            ones = work.tile([128, 1], BF16)
            nc.vector.tensor_copy(ones[:], nc.const_aps.aps[(BF16, 1.0)])
            with tc.tile_pool(name="psout", bufs=1, space="PSUM") as psout, \
                 tc.tile_pool(name="psden", bufs=1, space="PSUM") as psden:
                out_ps = psout.tile([QC, D], F32)
                den_ps = psden.tile([QC, 1], F32)
                for t in range(ntk):
                    nc.tensor.matmul(out_ps[:], num_tiles[t][:], v_sb[t][:],
                                     start=(t == 0), stop=(t == ntk - 1))
                for t in range(ntk):
                    nc.tensor.matmul(den_ps[:], num_tiles[t][:], ones[:, 0:1],
                                     start=(t == 0), stop=(t == ntk - 1))
                recip = work.tile([QC, 1], F32)
                nc.vector.reciprocal(recip[:], den_ps[:])
                out_sb = work.tile([QC, D], F32)
                nc.vector.tensor_scalar(out_sb[:], out_ps[:], recip[:, 0:1],
                                        None, ALU.mult)
                nc.sync.dma_start(out_ext[:], out_sb[:])
    return _fix_multiwait(nc)


def _direct_scores(nc, tc, work, kh_sb, qh_sb, wv, maskc, num_tiles,
                   Kpad, ntk, c_shift):
    """Exact tanh scores. scoresT[k, q] column-by-column via tiny matmuls."""
    with tc.tile_pool(name="pssc", bufs=1, space="PSUM") as pssc, \
         tc.tile_pool(name="tanhp", bufs=3) as tanhp:
        score_ps = [pssc.tile([128, QC], F32, name=f"sc{t}") for t in range(ntk)]
        for qi in range(QC):
            sum_t = tanhp.tile([H, Kpad], F32, tag="sum")
            nc.vector.tensor_scalar(sum_t[:], kh_sb[:], qh_sb[:, qi:qi + 1],
                                    None, ALU.add)
            tanh_t = tanhp.tile([H, Kpad], F32, tag="tanh")
            nc.scalar.activation(tanh_t[:], sum_t[:], AFT.Tanh)
            for t in range(ntk):
                nc.tensor.matmul(score_ps[t][:, qi:qi + 1],
                                 tanh_t[:, bass.ts(t, 128)], wv[:, 0:1],
                                 start=True, stop=True)
        for t in range(ntk):
            nc.scalar.activation(num_tiles[t][:], score_ps[t][:], AFT.Exp,
                                 bias=maskc[:, t:t + 1])


def _fourier_scores(nc, tc, work, kvq, qsc, maskrow, ident,
                    num_tiles, Kpad, ntk, omega, cm, c_shift, s_side):
    """Separable sin-feature scores, scores accumulated in [q, k] layout.

    kvq = [kh | qh] (fp32, [H, Kpad+QC]): one u/AND pass builds both sides'
    features. Feature j = 2*(m-1)+sc (sc: 0 sin, 1 cos; cos = frac+1/4 in the
    fixed-point domain). Features whose args fit ACT Sin's native [-3.1, 3.1]
    domain skip the fixed-point range reduction entirely.
    Score matmuls are emitted per feature chunk so PE overlaps the VE/ACT
    feature pipeline and stays HAM-warm. A rank-1 (ones x maskrow) term adds
    the softmax shift and the -30000 padding mask. exp -> num [q, k] ->
    PE-transpose per ktile -> numT tiles for the attention matmul."""
    NF = 2 * M_HARM
    W = Kpad + QC

    ones_row = work.tile([1, QC], BF16)
    nc.gpsimd.memset(ones_row[:], 1.0)

    def feat_params(j):
        m = j // 2 + 1
        is_cos = j % 2
        direct = (m * omega * s_side + (math.pi / 2 if is_cos else 0.0)) < 3.1
        return m, is_cos, direct

    CHUNK = 6
    with tc.tile_pool(name="featp", bufs=1) as featp, \
         tc.tile_pool(name="fwork", bufs=1) as fwp, \
         tc.tile_pool(name="pssc", bufs=1, space="PSUM") as pssc:
        feats = featp.tile([H, NF * W], BF16)   # [G | Q] per feature slice
        qfs = featp.tile([H, NF * QC], BF16)    # scaled Q-side copies

        chunks = [(c0, min(512, Kpad - c0)) for c0 in range(0, Kpad, 512)]
        ps_tiles = [pssc.tile([QC, 512], F32, name=f"sc{i}")
                    for i in range(len(chunks))]
        for i, (c0, wd) in enumerate(chunks):
            nc.tensor.matmul(ps_tiles[i][:, :wd], ones_row[:],
                             maskrow[:, c0:c0 + wd], start=True, stop=False)

        for j0 in range(0, NF, CHUNK):
            js = list(range(j0, min(j0 + CHUNK, NF)))
            red = [j for j in js if not feat_params(j)[2]]
            if red:
                wbuf = fwp.tile([H, len(red) * W], F32, tag="wband", bufs=2)
                for i, j in enumerate(red):
                    m, is_cos, _ = feat_params(j)
                    c_fix = m * omega / (2 * math.pi) * (1 << FB)
                    add_c = MAGIC + ((1 << FB) / 4.0 if is_cos else 0.0)
                    u = fwp.tile([H, W], F32, tag="u", bufs=3)
                    nc.vector.tensor_scalar(u[:], kvq[:], c_fix, add_c,
                                            ALU.mult, ALU.add)
                    nc.vector.tensor_scalar(
                        wbuf[:, i * W:(i + 1) * W].bitcast(I32),
                        u[:].bitcast(I32), KEEP_MASK, None, ALU.bitwise_and)
                # one batched Sin for the chunk's reduced features
                # (non-contiguous dest if chunk mixes direct features; the
                # reduced ones are emitted into their own slices one by one)
                if len(red) == len(js):
                    nc.scalar.activation(
                        feats[:, js[0] * W:(js[-1] + 1) * W], wbuf[:],
                        AFT.Sin, scale=ACT_SIN_SCALE, bias=ACT_SIN_BIAS)
                else:
                    for i, j in enumerate(red):
                        nc.scalar.activation(
                            feats[:, j * W:(j + 1) * W],
                            wbuf[:, i * W:(i + 1) * W],
                            AFT.Sin, scale=ACT_SIN_SCALE, bias=ACT_SIN_BIAS)
            for j in js:
                m, is_cos, direct = feat_params(j)
                if direct:
                    nc.scalar.activation(
                        feats[:, j * W:(j + 1) * W], kvq[:], AFT.Sin,
                        scale=m * omega,
                        bias=(math.pi / 2 if is_cos else 0.0))
            for j in js:
                nc.vector.tensor_scalar(qfs[:, bass.ts(j, QC)],
                                        feats[:, j * W + Kpad:(j + 1) * W],
                                        qsc[:, j:j + 1], None, ALU.mult)
            for j in js:
                pj = j ^ 1
                for i, (c0, wd) in enumerate(chunks):
                    nc.tensor.matmul(
                        ps_tiles[i][:, :wd],
                        qfs[:, bass.ts(pj, QC)],
                        feats[:, j * W + c0: j * W + c0 + wd],
                        start=False, stop=(j == NF - 1))

        num_qk = work.tile([QC, Kpad], BF16)
        for i, (c0, wd) in enumerate(chunks):
            nc.scalar.activation(num_qk[:, c0:c0 + wd], ps_tiles[i][:, :wd],
                                 AFT.Exp)

        # transpose num [q, k] -> numT tiles [k, q]
        with tc.tile_pool(name="pstr", bufs=2, space="PSUM") as pstr:
            for t in range(ntk):
                tr = pstr.tile([128, QC], BF16, tag="tr")
                nc.tensor.transpose(tr[:], num_qk[:, bass.ts(t, 128)], ident[:])
                nc.vector.tensor_copy(num_tiles[t][:], tr[:])


# ---------------------------------------------------------------------------
def kernel(q, k, v, valid_lens, w_q, w_k, w_v):
    q = np.asarray(q, np.float32)
    k = np.asarray(k, np.float32)
    v = np.asarray(v, np.float32)
    w_q = np.asarray(w_q, np.float32)
    w_k = np.asarray(w_k, np.float32)
    w_v = np.asarray(w_v, np.float32)
    vls = np.asarray(valid_lens).astype(np.int64)

    Kpad = int(min(K, ((int(vls.max()) + 127) // 128) * 128))
    ntk = Kpad // 128

    # score bound -> constant softmax shift (no max pass needed)
    c_shift = float(np.abs(w_v).sum()) + 0.5

    omega = cm = None
    s_side = 0.0
    if MODE == "fourier":
        qh = q.reshape(-1, D) @ w_q.T
        kh = k.reshape(-1, D) @ w_k.T
        S = float(np.abs(qh).max() + np.abs(kh).max()) * 1.02 + 1e-3
        s_side = float(max(np.abs(qh).max(), np.abs(kh).max())) * 1.02
        omega, cm, fit_err = _fit_fourier(S)

    key = (MODE, Kpad, None if omega is None else round(omega, 9),
           None if cm is None else round(float(cm[0]), 9), round(c_shift, 6))
    if key not in _GRAPH_CACHE:
        _GRAPH_CACHE[key] = _build_graph(Kpad, MODE, omega, cm, c_shift, s_side)
    nc = _GRAPH_CACHE[key]

    wqT = np.ascontiguousarray(w_q.T).astype(BF16NP)
    wkT = np.ascontiguousarray(w_k.T).astype(BF16NP)
    in_maps = []
    for c in range(N_CORES):
        b = c // 2
        qs = (c % 2) * QC
        vl = int(vls[b])
        im = {
            "kT": np.ascontiguousarray(k[b, :Kpad, :].T).astype(BF16NP),
            "v": np.ascontiguousarray(v[b, :Kpad, :]).astype(BF16NP),
            "qT": np.ascontiguousarray(q[b, qs:qs + QC, :].T).astype(BF16NP),
            "wqT": wqT, "wkT": wkT,
        }
        if MODE == "fourier":
            qscale = np.empty((H, 2 * M_HARM), np.float32)
            for j in range(2 * M_HARM):
                qscale[:, j] = w_v * cm[j // 2]
            im["qscale"] = qscale
            maskrow = np.full((1, Kpad), PAD_BIAS, np.float32)
            maskrow[0, :vl] = -c_shift
            im["maskrow"] = maskrow.astype(BF16NP)
            im["ident"] = np.eye(128, dtype=BF16NP)
        else:
            maskc = np.full((128, ntk), PAD_BIAS, np.float32)
            for t in range(ntk):
                n_valid = min(128, max(0, vl - t * 128))
                maskc[:n_valid, t] = -c_shift
            im["maskc"] = maskc
            im["wv"] = w_v.reshape(H, 1).astype(np.float32)
        in_maps.append(im)

    res = run_bass_kernel_spmd(nc, in_maps, core_ids=list(range(N_CORES)))
    out = np.empty((B, Q, D), np.float32)
    for c in range(N_CORES):
        b = c // 2
        qs = (c % 2) * QC
        out[b, qs:qs + QC, :] = res.results[c]["out"]
    return out


# revision 12
# speedup vs baseline: 5.5644x; 1.0066x over previous
"""Additive (Bahdanau) attention on 8 TRN2 NeuronCores.

Reference computation (B=4, Q=256, K=1024, D=512, H=128):
    qh = q @ w_q.T                      [B,Q,H]
    kh = k @ w_k.T                      [B,K,H]
    scores[b,q,k] = sum_h w_v[h] * tanh(qh[b,q,h] + kh[b,k,h])
    scores masked to -1e6 for k >= valid_lens[b]; softmax over k; out = attn @ v.

Sharding: core c handles batch b = c//2, query rows [(c%2)*128, +128) with ALL
of that batch's valid keys (padded to Kpad = ceil(max(vl)/128)*128). Each core
owns complete softmax rows -> no collectives; host just concatenates.

Masked keys are exact zeros after softmax in fp32 (exp(-1e6) underflows), so
computing only the first Kpad keys reproduces the reference bit-for-bit in
structure. Scores are bounded by sum|w_v|, so softmax needs no max-reduction:
exp(score - C) with constant C is stable.

Score modes:
  direct : tanh computed exactly on ScalarE; H-reduction via per-(q, ktile)
           matmuls with the tanh tile as stationary and w_v as moving.
  fourier: tanh(a+b) ~= sum_m c_m sin(m*w*(a+b)) expanded by the angle-sum
           identity into rank-2M separable features -> the whole score tensor
           becomes TensorE matmuls with contraction dim 2M*128. sin args are
           range-reduced to [-pi, pi] with a fixed-point magic-round + bitmask
           trick (ACT Sin diverges outside one period).
"""
import math
import os
import numpy as np
import ml_dtypes

import concourse.bass as bass
import concourse.mybir as mybir
from concourse.tile import TileContext
from concourse.bass_utils import run_bass_kernel_spmd

F32 = mybir.dt.float32
BF16 = mybir.dt.bfloat16
I32 = mybir.dt.int32
AFT = mybir.ActivationFunctionType
ALU = mybir.AluOpType
BF16NP = ml_dtypes.bfloat16

B, Q, K, D, H = 4, 256, 1024, 512, 128
QC = 128           # query rows per core
N_CORES = 8
MODE = os.environ.get("KMODE", "direct")   # "direct" | "fourier"
M_HARM = int(os.environ.get("KM", "12"))  # fourier harmonics
FB = 14            # fixed-point fractional bits for sin range reduction
MAGIC = 1.5 * 2.0**23
KEEP_MASK = 0x4B000000 | ((1 << FB) - 1)
ACT_SIN_SCALE = -2.0 * math.pi / (1 << FB)
ACT_SIN_BIAS = 2.0 * math.pi * (2.0**23) / (1 << FB) + math.pi
PAD_BIAS = -30000.0  # exp(score + PAD_BIAS) == 0 exactly for padded keys

_GRAPH_CACHE = {}


# ---------------------------------------------------------------------------
# BIR post-pass: this container's walrus accepts at most 1 sync-wait per
# instruction (2 on EventSemaphore). Tile sometimes emits more (notably the
# kernel-tail drain). Hoist the excess onto standalone EventSemaphores.
def _fix_multiwait(nc):
    ctr = 0
    for f in nc.m.functions:
        for bb in f.blocks:
            ins_list = bb.instructions
            if not any(
                len(i.sync_info.on_wait)
                > (2 if isinstance(i, mybir.InstEventSemaphore) else 1)
                for i in ins_list
                if getattr(i, "sync_info", None) is not None
            ):
                continue
            new_list = []
            for inst in ins_list:
                si = getattr(inst, "sync_info", None)
                if si is not None:
                    waits = list(si.on_wait)
                    cap = 2 if isinstance(inst, mybir.InstEventSemaphore) else 1
                    if len(waits) > cap:
                        extra = waits[cap:]
                        for kk in range(0, len(extra), 2):
                            es = mybir.InstEventSemaphore(
                                name=f"waitfix_{ctr}", engine=inst.engine
                            )
                            ctr += 1
                            es.sync_info = mybir.SyncInfo(
                                on_wait=extra[kk : kk + 2], on_update=[]
                            )
                            new_list.append(es)
                        inst.sync_info = mybir.SyncInfo(
                            on_wait=waits[:cap], on_update=list(si.on_update)
                        )
                new_list.append(inst)
            bb.instructions = new_list
    return nc


def _register_const(nc, val, dtype=F32):
    """Const-AP so activation(bias=<float>) lowers; barrier mirrors Bass init."""
    if (dtype, val) in nc.const_aps.aps:
        return
    t = nc.alloc_sbuf_tensor(f"constap-{len(nc.const_aps.aps)}", [128, 1], dtype)
    nc.gpsimd.memset(t.ap(), val)
    nc.const_aps.aps[(dtype, val)] = t.ap()
    nc.all_engine_barrier()


def _fit_fourier(S, sigma=None):
    """Weighted lstsq fit tanh(s) ~= sum_m c_m sin(m*omega*s) on [-S, S].
    Weight follows the empirical distribution of s = qh+kh (approx normal
    with std sigma) plus a uniform floor, so error lands where data lives."""
    P = 1.16 * S
    omega = math.pi / P
    s = np.linspace(-S, S, 20001)
    A = np.sin(np.outer(s, omega * np.arange(1, M_HARM + 1)))
    w = np.ones_like(s)
    if sigma is not None:
        w = np.sqrt(np.exp(-0.5 * (s / sigma) ** 2) + 0.02)
    c, *_ = np.linalg.lstsq(A * w[:, None], np.tanh(s) * w, rcond=None)
    err = float(np.abs(A @ c - np.tanh(s)).max())
    return omega, c.astype(np.float64), err


# ---------------------------------------------------------------------------
def _build_graph(Kpad, mode, omega, cm, c_shift, s_side=0.0):
    ntk = Kpad // 128
    nc = bass.Bass()
    _register_const(nc, ACT_SIN_BIAS)
    _register_const(nc, math.pi / 2)

    kT_ext = nc.declare_dram_parameter("kT", [D, Kpad], BF16, isOutput=False)
    v_ext = nc.declare_dram_parameter("v", [Kpad, D], BF16, isOutput=False)
    qT_ext = nc.declare_dram_parameter("qT", [D, QC], BF16, isOutput=False)
    wqT_ext = nc.declare_dram_parameter("wqT", [D, H], BF16, isOutput=False)
    wkT_ext = nc.declare_dram_parameter("wkT", [D, H], BF16, isOutput=False)
    out_ext = nc.declare_dram_parameter("out", [QC, D], F32, isOutput=True)
    if mode == "fourier":
        qsc_ext = nc.declare_dram_parameter("qscale", [H, 2 * M_HARM], F32, isOutput=False)
        maskr_ext = nc.declare_dram_parameter("maskrow", [1, Kpad], BF16, isOutput=False)
        ident_ext = nc.declare_dram_parameter("ident", [128, 128], BF16, isOutput=False)
    else:
        maskc_ext = nc.declare_dram_parameter("maskc", [128, ntk], F32, isOutput=False)
        wv_ext = nc.declare_dram_parameter("wv", [H, 1], F32, isOutput=False)

    with TileContext(nc) as tc:
        with tc.tile_pool(name="io", bufs=1) as io, \
             tc.tile_pool(name="work", bufs=1) as work:
            # ---- load inputs. Small tensors on the gpsimd (SWDGE) queue,
            # big ones on sync (HWDGE); weights/kT first, v last (only the
            # attention tail needs it).
            wqT = [io.tile([128, H], BF16, name=f"wqT{i}") for i in range(4)]
            wkT = [io.tile([128, H], BF16, name=f"wkT{i}") for i in range(4)]
            qT = [io.tile([128, QC], BF16, name=f"qT{i}") for i in range(4)]
            for i in range(4):
                nc.scalar.dma_start(wkT[i][:], wkT_ext[bass.ts(i, 128), :])
            for i in range(4):
                nc.gpsimd.dma_start(qT[i][:], qT_ext[bass.ts(i, 128), :])
                nc.gpsimd.dma_start(wqT[i][:], wqT_ext[bass.ts(i, 128), :])
            kT = [io.tile([128, Kpad], BF16, name=f"kT{i}") for i in range(4)]
            for i in range(4):
                nc.sync.dma_start(kT[i][:], kT_ext[bass.ts(i, 128), :])
            if mode == "fourier":
                qsc = io.tile([H, 2 * M_HARM], F32)
                nc.gpsimd.dma_start(qsc[:], qsc_ext[:])
                maskrow = io.tile([1, Kpad], BF16)
                nc.gpsimd.dma_start(maskrow[:], maskr_ext[:])
                ident = io.tile([128, 128], BF16)
                nc.gpsimd.dma_start(ident[:], ident_ext[:])
            else:
                maskc = io.tile([128, ntk], F32)
                nc.gpsimd.dma_start(maskc[:], maskc_ext[:])
                wv = io.tile([H, 1], F32)
                nc.gpsimd.dma_start(wv[:], wv_ext[:])
            v_sb = [io.tile([128, D], BF16, name=f"v{t}") for t in range(ntk)]
            for t in range(ntk):
                nc.sync.dma_start(v_sb[t][:], v_ext[bass.ts(t, 128), :])

            # ---- projections -> kvq = [kh | qh] fp32 (fourier) or
            # kh_sb/qh_sb (direct)
            if mode == "fourier":
                kvq = work.tile([H, Kpad + QC], F32)
                kh_sb = kvq[:, :Kpad]
                qh_sb = kvq[:, Kpad:]
            else:
                kh_t = work.tile([H, Kpad], F32)
                qh_t = work.tile([H, QC], F32)
                kh_sb = kh_t[:]
                qh_sb = qh_t[:]
            with tc.tile_pool(name="psproj", bufs=2, space="PSUM") as psproj:
                for c0 in range(0, Kpad, 512):
                    w = min(512, Kpad - c0)
                    ps = psproj.tile([128, 512], F32, tag="proj")
                    for dt_ in range(4):
                        nc.tensor.matmul(ps[:, :w], wkT[dt_][:],
                                         kT[dt_][:, c0:c0 + w],
                                         start=(dt_ == 0), stop=(dt_ == 3))
                    nc.vector.tensor_copy(kh_sb[:, c0:c0 + w], ps[:, :w])
                ps = psproj.tile([128, 512], F32, tag="proj")
                for dt_ in range(4):
                    nc.tensor.matmul(ps[:, :QC], wqT[dt_][:], qT[dt_][:],
                                     start=(dt_ == 0), stop=(dt_ == 3))
                nc.vector.tensor_copy(qh_sb[:], ps[:, :QC])

            num_tiles = [work.tile([128, QC], BF16, name=f"num{t}") for t in range(ntk)]

            if mode == "fourier":
                _fourier_scores(nc, tc, work, kvq, qsc, maskrow, ident,
                                num_tiles, Kpad, ntk, omega, cm, c_shift, s_side)
            else:
                _direct_scores(nc, tc, work, kh_sb, qh_sb, wv, maskc,
                               num_tiles, Kpad, ntk, c_shift)

            # ---- attn @ v and denominator
            ones = work.tile([128, 1], BF16)
            nc.vector.tensor_copy(ones[:], nc.const_aps.aps[(BF16, 1.0)])
            with tc.tile_pool(name="psout", bufs=1, space="PSUM") as psout, \
                 tc.tile_pool(name="psden", bufs=1, space="PSUM") as psden:
                out_ps = psout.tile([QC, D], F32)
                den_ps = psden.tile([QC, 1], F32)
                for t in range(ntk):
                    nc.tensor.matmul(out_ps[:], num_tiles[t][:], v_sb[t][:],
                                     start=(t == 0), stop=(t == ntk - 1))
                for t in range(ntk):
                    nc.tensor.matmul(den_ps[:], num_tiles[t][:], ones[:, 0:1],
                                     start=(t == 0), stop=(t == ntk - 1))
                recip = work.tile([QC, 1], F32)
                nc.vector.reciprocal(recip[:], den_ps[:])
                out_sb = work.tile([QC, D], F32)
                nc.vector.tensor_scalar(out_sb[:], out_ps[:], recip[:, 0:1],
                                        None, ALU.mult)
                nc.sync.dma_start(out_ext[:], out_sb[:])
    return _fix_multiwait(nc)


def _direct_scores(nc, tc, work, kh_sb, qh_sb, wv, maskc, num_tiles,
                   Kpad, ntk, c_shift):
    """Exact tanh scores. scoresT[k, q] column-by-column via tiny matmuls."""
    with tc.tile_pool(name="pssc", bufs=1, space="PSUM") as pssc, \
         tc.tile_pool(name="tanhp", bufs=3) as tanhp:
        score_ps = [pssc.tile([128, QC], F32, name=f"sc{t}") for t in range(ntk)]
        for qi in range(QC):
            sum_t = tanhp.tile([H, Kpad], F32, tag="sum")
            nc.vector.tensor_scalar(sum_t[:], kh_sb[:], qh_sb[:, qi:qi + 1],
                                    None, ALU.add)
            tanh_t = tanhp.tile([H, Kpad], F32, tag="tanh")
            nc.scalar.activation(tanh_t[:], sum_t[:], AFT.Tanh)
            for t in range(ntk):
                nc.tensor.matmul(score_ps[t][:, qi:qi + 1],
                                 tanh_t[:, bass.ts(t, 128)], wv[:, 0:1],
                                 start=True, stop=True)
        for t in range(ntk):
            nc.scalar.activation(num_tiles[t][:], score_ps[t][:], AFT.Exp,
                                 bias=maskc[:, t:t + 1])


def _fourier_scores(nc, tc, work, kvq, qsc, maskrow, ident,
                    num_tiles, Kpad, ntk, omega, cm, c_shift, s_side):
    """Separable sin-feature scores, scores accumulated in [q, k] layout.

    kvq = [kh | qh] (fp32, [H, Kpad+QC]): one u/AND pass builds both sides'
    features. Feature j = 2*(m-1)+sc (sc: 0 sin, 1 cos; cos = frac+1/4 in the
    fixed-point domain). Features whose args fit ACT Sin's native [-3.1, 3.1]
    domain skip the fixed-point range reduction entirely.
    Score matmuls are emitted per feature chunk so PE overlaps the VE/ACT
    feature pipeline and stays HAM-warm. A rank-1 (ones x maskrow) term adds
    the softmax shift and the -30000 padding mask. exp -> num [q, k] ->
    PE-transpose per ktile -> numT tiles for the attention matmul."""
    NF = 2 * M_HARM
    W = Kpad + QC

    ones_row = work.tile([1, QC], BF16)
    nc.gpsimd.memset(ones_row[:], 1.0)

    def feat_params(j):
        m = j // 2 + 1
        is_cos = j % 2
        direct = (m * omega * s_side + (math.pi / 2 if is_cos else 0.0)) < 3.1
        return m, is_cos, direct

    CHUNK = 6
    with tc.tile_pool(name="featp", bufs=1) as featp, \
         tc.tile_pool(name="fwork", bufs=1) as fwp, \
         tc.tile_pool(name="pssc", bufs=1, space="PSUM") as pssc:
        feats = featp.tile([H, NF * W], BF16)   # [G | Q] per feature slice
        qfs = featp.tile([H, NF * QC], BF16)    # scaled Q-side copies

        chunks = [(c0, min(512, Kpad - c0)) for c0 in range(0, Kpad, 512)]
        ps_tiles = [pssc.tile([QC, 512], F32, name=f"sc{i}")
                    for i in range(len(chunks))]
        for i, (c0, wd) in enumerate(chunks):
            nc.tensor.matmul(ps_tiles[i][:, :wd], ones_row[:],
                             maskrow[:, c0:c0 + wd], start=True, stop=False)

        for j0 in range(0, NF, CHUNK):
            js = list(range(j0, min(j0 + CHUNK, NF)))
            red = [j for j in js if not feat_params(j)[2]]
            if red:
                wbuf = fwp.tile([H, len(red) * W], F32, tag="wband", bufs=2)
                for i, j in enumerate(red):
                    m, is_cos, _ = feat_params(j)
                    c_fix = m * omega / (2 * math.pi) * (1 << FB)
                    add_c = MAGIC + ((1 << FB) / 4.0 if is_cos else 0.0)
                    u = fwp.tile([H, W], F32, tag="u", bufs=3)
                    if j % 3 == 2:  # offload ~1/3 of the u affines to ACT
                        nc.scalar.activation(u[:], kvq[:], AFT.Copy,
                                             scale=c_fix, bias=add_c)
                    else:
                        nc.vector.tensor_scalar(u[:], kvq[:], c_fix, add_c,
                                                ALU.mult, ALU.add)
                    nc.vector.tensor_scalar(
                        wbuf[:, i * W:(i + 1) * W].bitcast(I32),
                        u[:].bitcast(I32), KEEP_MASK, None, ALU.bitwise_and)
                # one batched Sin for the chunk's reduced features
                # (non-contiguous dest if chunk mixes direct features; the
                # reduced ones are emitted into their own slices one by one)
                if len(red) == len(js):
                    nc.scalar.activation(
                        feats[:, js[0] * W:(js[-1] + 1) * W], wbuf[:],
                        AFT.Sin, scale=ACT_SIN_SCALE, bias=ACT_SIN_BIAS)
                else:
                    for i, j in enumerate(red):
                        nc.scalar.activation(
                            feats[:, j * W:(j + 1) * W],
                            wbuf[:, i * W:(i + 1) * W],
                            AFT.Sin, scale=ACT_SIN_SCALE, bias=ACT_SIN_BIAS)
            for j in js:
                m, is_cos, direct = feat_params(j)
                if direct:
                    nc.scalar.activation(
                        feats[:, j * W:(j + 1) * W], kvq[:], AFT.Sin,
                        scale=m * omega,
                        bias=(math.pi / 2 if is_cos else 0.0))
            for j in js:
                nc.vector.tensor_scalar(qfs[:, bass.ts(j, QC)],
                                        feats[:, j * W + Kpad:(j + 1) * W],
                                        qsc[:, j:j + 1], None, ALU.mult)
            for j in js:
                pj = j ^ 1
                for i, (c0, wd) in enumerate(chunks):
                    nc.tensor.matmul(
                        ps_tiles[i][:, :wd],
                        qfs[:, bass.ts(pj, QC)],
                        feats[:, j * W + c0: j * W + c0 + wd],
                        start=False, stop=(j == NF - 1))

        num_qk = work.tile([QC, Kpad], BF16)
        for i, (c0, wd) in enumerate(chunks):
            nc.scalar.activation(num_qk[:, c0:c0 + wd], ps_tiles[i][:, :wd],
                                 AFT.Exp)

        # transpose num [q, k] -> numT tiles [k, q]
        with tc.tile_pool(name="pstr", bufs=2, space="PSUM") as pstr:
            for t in range(ntk):
                tr = pstr.tile([128, QC], BF16, tag="tr")
                nc.tensor.transpose(tr[:], num_qk[:, bass.ts(t, 128)], ident[:])
                nc.vector.tensor_copy(num_tiles[t][:], tr[:])


# ---------------------------------------------------------------------------
def kernel(q, k, v, valid_lens, w_q, w_k, w_v):
    q = np.asarray(q, np.float32)
    k = np.asarray(k, np.float32)
    v = np.asarray(v, np.float32)
    w_q = np.asarray(w_q, np.float32)
    w_k = np.asarray(w_k, np.float32)
    w_v = np.asarray(w_v, np.float32)
    vls = np.asarray(valid_lens).astype(np.int64)

    Kpad = int(min(K, ((int(vls.max()) + 127) // 128) * 128))
    ntk = Kpad // 128

    # score bound -> constant softmax shift (no max pass needed)
    c_shift = float(np.abs(w_v).sum()) + 0.5

    omega = cm = None
    s_side = 0.0
    if MODE == "fourier":
        qh = q.reshape(-1, D) @ w_q.T
        kh = k.reshape(-1, D) @ w_k.T
        S = float(np.abs(qh).max() + np.abs(kh).max()) * 1.02 + 1e-3
        s_side = float(max(np.abs(qh).max(), np.abs(kh).max())) * 1.02
        sigma = float(np.sqrt(qh.var() + kh.var()))
        omega, cm, fit_err = _fit_fourier(S, sigma)

    key = (MODE, Kpad, None if omega is None else round(omega, 9),
           None if cm is None else round(float(cm[0]), 9), round(c_shift, 6))
    if key not in _GRAPH_CACHE:
        _GRAPH_CACHE[key] = _build_graph(Kpad, MODE, omega, cm, c_shift, s_side)
    nc = _GRAPH_CACHE[key]

    wqT = np.ascontiguousarray(w_q.T).astype(BF16NP)
    wkT = np.ascontiguousarray(w_k.T).astype(BF16NP)
    in_maps = []
    for c in range(N_CORES):
        b = c // 2
        qs = (c % 2) * QC
        vl = int(vls[b])
        im = {
            "kT": np.ascontiguousarray(k[b, :Kpad, :].T).astype(BF16NP),
            "v": np.ascontiguousarray(v[b, :Kpad, :]).astype(BF16NP),
            "qT": np.ascontiguousarray(q[b, qs:qs + QC, :].T).astype(BF16NP),
            "wqT": wqT, "wkT": wkT,
        }
        if MODE == "fourier":
            qscale = np.empty((H, 2 * M_HARM), np.float32)
            for j in range(2 * M_HARM):
                qscale[:, j] = w_v * cm[j // 2]
            im["qscale"] = qscale
            maskrow = np.full((1, Kpad), PAD_BIAS, np.float32)
            maskrow[0, :vl] = -c_shift
            im["maskrow"] = maskrow.astype(BF16NP)
            im["ident"] = np.eye(128, dtype=BF16NP)
        else:
            maskc = np.full((128, ntk), PAD_BIAS, np.float32)
            for t in range(ntk):
                n_valid = min(128, max(0, vl - t * 128))
                maskc[:n_valid, t] = -c_shift
            im["maskc"] = maskc
            im["wv"] = w_v.reshape(H, 1).astype(np.float32)
        in_maps.append(im)

    res = run_bass_kernel_spmd(nc, in_maps, core_ids=list(range(N_CORES)))
    out = np.empty((B, Q, D), np.float32)
    for c in range(N_CORES):
        b = c // 2
        qs = (c % 2) * QC
        out[b, qs:qs + QC, :] = res.results[c]["out"]
    return out
